# revision 8
# baseline (speedup 1.0000x reference)
"""Trainium2 Bass kernel for nn_Attention2D (sparse_attention).

Computes, per batch element b (data-parallel over 8 NeuronCores):
  sig    = query @ Wsig.T + bsig -> per-head Gaussian widths (s1, s2)
  target = 1/(2*pi*s1*s2) * exp(-dy/(2*s1^2) - dx/(2*s2^2))   [H, L, L]
  q,k,v  = projections; attn = softmax(q k^T / sqrt(dh))      [H, L, L]
  out    = attn @ v                                            [L, D]

Key device-side structure per core:
  - query/key/value transposed on PE (identity matmuls) to feed projections.
  - q_projT / k_projT kept head-major on partitions so K=64 attention
    matmuls for even/odd heads land on disjoint PE row groups (row tiling).
  - scores computed twice (attn [l,s] and attnT [s,l]); exp on ACT gives
    E (for p, with free running-sum accum_out = softmax denominator Z) and
    ET (transposed weights feeding the out = E @ v matmul as lhsT).
  - target exponent u = a_l*dy + b_l*dx realized exactly as a K=64 matmul
    against indicator-selected rows of dy/dx (dedup of dy rows, host-side),
    avoiding any catastrophic cancellation for huge a (up to ~4e9).
"""

import math
import os
import sys

import numpy as np

for _p in ("/opt/trn_rl_repo", "/root/.axon_site/_ro/trn_rl_repo"):
    if os.path.isdir(_p) and _p not in sys.path:
        sys.path.insert(0, _p)

import concourse.bacc as bacc
import concourse.mybir as mybir
import concourse.tile as tile
from concourse.bass_utils import run_bass_kernel_spmd

F32 = mybir.dt.float32
F32R = mybir.dt.float32r
AF = mybir.ActivationFunctionType
OP = mybir.AluOpType

B, L, D, H = 8, 1024, 512, 8
DH = D // H            # 64
NV = 32                # max distinct dy/dx row groups per axis
SCALE = 1.0 / math.sqrt(DH)
LN3 = math.log(3.0)
LOG2PI = math.log(2.0 * math.pi)
N_CORES = 8

_module_cache = {}
last_results = None  # BassKernelResults of the most recent device run


def _build_module():
    nc = bacc.Bacc("TRN2", target_bir_lowering=False, debug=False)

    q_ap = nc.dram_tensor("query", [L, D], F32, kind="ExternalInput").ap()
    k_ap = nc.dram_tensor("key", [L, D], F32, kind="ExternalInput").ap()
    v_ap = nc.dram_tensor("value", [L, D], F32, kind="ExternalInput").ap()
    wqT_ap = nc.dram_tensor("WqT", [D, D], F32, kind="ExternalInput").ap()
    wkT_ap = nc.dram_tensor("WkT", [D, D], F32, kind="ExternalInput").ap()
    wvT_ap = nc.dram_tensor("WvT", [D, D], F32, kind="ExternalInput").ap()
    wsT_ap = nc.dram_tensor("WsigT", [D, 2 * H], F32, kind="ExternalInput").ap()
    bq_ap = nc.dram_tensor("bqT", [128, 4], F32, kind="ExternalInput").ap()
    bk_ap = nc.dram_tensor("bkT", [128, 4], F32, kind="ExternalInput").ap()
    bv_ap = nc.dram_tensor("bvR", [1, D], F32, kind="ExternalInput").ap()
    bs_ap = nc.dram_tensor("bsR", [1, 2 * H], F32, kind="ExternalInput").ap()
    Y_ap = nc.dram_tensor("Ymask", [128, 8 * NV], F32, kind="ExternalInput").ap()
    X_ap = nc.dram_tensor("Xmask", [128, 8 * NV], F32, kind="ExternalInput").ap()
    F2_ap = nc.dram_tensor("F2", [128, L], F32, kind="ExternalInput").ap()
    id_ap = nc.dram_tensor("ident", [128, 128], F32, kind="ExternalInput").ap()

    out_ap = nc.dram_tensor("out", [L, D], F32, kind="ExternalOutput").ap()
    p_ap = nc.dram_tensor("p", [H, L, L], F32, kind="ExternalOutput").ap()
    t_ap = nc.dram_tensor("target", [H, L, L], F32, kind="ExternalOutput").ap()

    with tile.TileContext(nc) as tc:
        with tc.tile_pool(name="pers", bufs=1) as pers:
            qpH = [pers.tile([128, L], F32R, name=f"qpH{c}") for c in range(4)]
            qpL = [pers.tile([128, L], F32R, name=f"qpL{c}") for c in range(4)]
            kpH = [pers.tile([128, L], F32R, name=f"kpH{c}") for c in range(4)]
            kpL = [pers.tile([128, L], F32R, name=f"kpL{c}") for c in range(4)]
            v_sb = [pers.tile([128, D], F32, name=f"vsb{st}") for st in range(8)]
            osb = [pers.tile([128, D], F32, name=f"osb{lt}") for lt in range(8)]
            F2r_sb = pers.tile([128, L], F32R, name="F2r_sb")
            sig_sb = pers.tile([128, 128], F32, name="sig_sb")
            ab_sb = pers.tile([128, 128], F32, name="ab_sb")
            abH_sb = pers.tile([128, 128], F32R, name="abH_sb")
            abL_sb = pers.tile([128, 128], F32R, name="abL_sb")
            lc_sb = pers.tile([128, 64], F32, name="lc_sb")
            z_all = pers.tile([128, 64], F32, name="z_all")
            rz_sb = pers.tile([128, 64], F32, name="rz_sb")
            Y_sb = pers.tile([128, 8 * NV], F32, name="Y_sb")
            X_sb = pers.tile([128, 8 * NV], F32, name="X_sb")
            F2_sb = pers.tile([128, L], F32, name="F2_sb")
            id_sb = pers.tile([128, 128], F32, name="id_sb")
            ones_sb = pers.tile([1, 128], F32, name="ones_sb")
            bq_sb = pers.tile([128, 4], F32, name="bq_sb")
            bk_sb = pers.tile([128, 4], F32, name="bk_sb")
            bv_sb = pers.tile([1, D], F32, name="bv_sb")
            bs_sb = pers.tile([1, 2 * H], F32, name="bs_sb")

            nc.sync.dma_start(Y_sb[:], Y_ap[:])
            nc.sync.dma_start(X_sb[:], X_ap[:])
            nc.sync.dma_start(F2_sb[:], F2_ap[:])
            nc.sync.dma_start(id_sb[:], id_ap[:])
            nc.sync.dma_start(bq_sb[:], bq_ap[:])
            nc.sync.dma_start(bk_sb[:], bk_ap[:])
            nc.sync.dma_start(bv_sb[:], bv_ap[:])
            nc.sync.dma_start(bs_sb[:], bs_ap[:])
            nc.gpsimd.memset(ones_sb[:], 1.0)
            nc.gpsimd.memset(z_all[:], 0.0)

            # ---- Phase A+B: input transposes + projections ----
            with (
                tc.tile_pool(name="pab", bufs=1) as pab,
                tc.tile_pool(name="psab", bufs=2, space="PSUM") as psab,
            ):
                wq_sb = [pab.tile([128, D], F32, name=f"wq{kc}") for kc in range(4)]
                wk_sb = [pab.tile([128, D], F32, name=f"wk{kc}") for kc in range(4)]
                wv_sb = [pab.tile([128, D], F32, name=f"wv{kc}") for kc in range(4)]
                ws_sb = [pab.tile([128, 2 * H], F32, name=f"ws{kc}") for kc in range(4)]
                for kc in range(4):
                    sl = slice(kc * 128, (kc + 1) * 128)
                    nc.sync.dma_start(wq_sb[kc][:], wqT_ap[sl, :])
                    nc.sync.dma_start(wk_sb[kc][:], wkT_ap[sl, :])
                    nc.sync.dma_start(wv_sb[kc][:], wvT_ap[sl, :])
                    nc.sync.dma_start(ws_sb[kc][:], wsT_ap[sl, :])

                xT = {}
                for nm in ("q", "k", "v"):
                    for kc in range(4):
                        xT[(nm, kc)] = pab.tile([128, L], F32, name=f"{nm}T{kc}")
                for nm, ap in (("q", q_ap), ("k", k_ap), ("v", v_ap)):
                    for lt in range(8):
                        raw = pab.tile([128, D], F32, name="raw", tag="raw", bufs=3)
                        nc.sync.dma_start(raw[:], ap[lt * 128:(lt + 1) * 128, :])
                        tp = psab.tile([128, 512], F32, name="tp", tag="tp", bufs=2)
                        for kc in range(4):
                            nc.tensor.transpose(
                                tp[:, kc * 128:(kc + 1) * 128],
                                raw[:, kc * 128:(kc + 1) * 128],
                                id_sb[:],
                            )
                        for kc in range(4):
                            nc.vector.tensor_copy(
                                xT[(nm, kc)][:, lt * 128:(lt + 1) * 128],
                                tp[:, kc * 128:(kc + 1) * 128],
                            )

                # q_projT / k_projT: [d_out, l], head-major on partitions
                for c in range(4):
                    for half in range(2):
                        hsl = slice(half * 512, (half + 1) * 512)
                        pq = psab.tile([128, 512], F32, name="pq", tag="pq", bufs=2)
                        for kc in range(4):
                            nc.tensor.matmul(
                                pq[:], wq_sb[kc][:, c * 128:(c + 1) * 128],
                                xT[("q", kc)][:, hsl],
                                start=(kc == 0), stop=(kc == 3),
                            )
                        nc.vector.tensor_scalar(
                            qpH[c][:, hsl], pq[:], bq_sb[:, c:c + 1], None, OP.add)
                        nc.vector.scalar_tensor_tensor(
                            qpL[c][:, hsl], pq[:], bq_sb[:, c:c + 1],
                            qpH[c][:, hsl], OP.add, OP.subtract)
                        pk = psab.tile([128, 512], F32, name="pk", tag="pq", bufs=2)
                        for kc in range(4):
                            nc.tensor.matmul(
                                pk[:], wk_sb[kc][:, c * 128:(c + 1) * 128],
                                xT[("k", kc)][:, hsl],
                                start=(kc == 0), stop=(kc == 3),
                            )
                        nc.vector.tensor_scalar(
                            kpH[c][:, hsl], pk[:], bk_sb[:, c:c + 1], None, OP.add)
                        nc.vector.scalar_tensor_tensor(
                            kpL[c][:, hsl], pk[:], bk_sb[:, c:c + 1],
                            kpH[c][:, hsl], OP.add, OP.subtract)

                # v_proj: natural [s, d] layout (+ bias via rank-1 matmul)
                for st in range(8):
                    pv = psab.tile([128, 512], F32, name="pv", tag="pq", bufs=2)
                    for kc in range(4):
                        nc.tensor.matmul(
                            pv[:], xT[("v", kc)][:, st * 128:(st + 1) * 128],
                            wv_sb[kc][:], start=(kc == 0), stop=False,
                        )
                    nc.tensor.matmul(pv[:], ones_sb[:1, :], bv_sb[:1, :],
                                     start=False, stop=True)
                    nc.vector.tensor_copy(v_sb[st][:], pv[:])

                # sig = query @ Wsig.T + bsig: [l, 16] per l-tile
                for lt in range(8):
                    psg = psab.tile([128, 2 * H], F32, name="psg", tag="psg", bufs=2)
                    for kc in range(4):
                        nc.tensor.matmul(
                            psg[:], xT[("q", kc)][:, lt * 128:(lt + 1) * 128],
                            ws_sb[kc][:], start=(kc == 0), stop=False,
                        )
                    nc.tensor.matmul(psg[:], ones_sb[:1, :], bs_sb[:1, :],
                                     start=False, stop=True)
                    nc.vector.tensor_copy(
                        sig_sb[:, lt * 16:(lt + 1) * 16], psg[:])

            # ---- Phase C: sigma transforms ----
            # s = 3^(sigmoid(5x)+1e-5) - 1 ; a/b = 1/(2 s^2) ;
            # logc = -(log(2 pi) + log s1 + log s2)
            with tc.tile_pool(name="post", bufs=1) as post:
                sg = post.tile([128, 128], F32, name="sg")
                nc.scalar.activation(sg[:], sig_sb[:], AF.Sigmoid, scale=5.0)
                # s = expm1(z), z = ln3*(sigmoid+1e-5).  exp(z)-1 cancels
                # catastrophically for tiny z (s ~ 1e-5 drives the largest
                # target values), so blend a Taylor branch below z=0.12.
                zt = post.tile([128, 128], F32, name="zt")
                nc.vector.tensor_scalar(zt[:], sg[:], LN3, LN3 * 1e-5,
                                        OP.mult, OP.add)
                ez = post.tile([128, 128], F32, name="ez")
                nc.scalar.activation(ez[:], zt[:], AF.Exp)
                s12 = post.tile([128, 128], F32, name="s12")
                nc.vector.tensor_scalar(s12[:], ez[:], -1.0, None, OP.add)
                hh = post.tile([128, 128], F32, name="hh")
                tmp = post.tile([128, 128], F32, name="tmp")
                nc.vector.tensor_scalar(hh[:], zt[:], 1.0 / 120, 1.0 / 24,
                                        OP.mult, OP.add)
                for cst in (1.0 / 6, 0.5, 1.0):
                    nc.vector.tensor_tensor(tmp[:], hh[:], zt[:], OP.mult)
                    nc.vector.tensor_scalar(hh[:], tmp[:], cst, None, OP.add)
                st_ = post.tile([128, 128], F32, name="st_")
                nc.vector.tensor_tensor(st_[:], hh[:], zt[:], OP.mult)
                msk = post.tile([128, 128], mybir.dt.uint8, name="msk")
                nc.vector.tensor_scalar(msk[:], zt[:], 0.12, None, OP.is_lt)
                nc.vector.copy_predicated(s12[:], msk[:], st_[:])
                ssq = post.tile([128, 128], F32, name="ssq")
                nc.vector.tensor_tensor(ssq[:], s12[:], s12[:], OP.mult)
                rec = post.tile([128, 128], F32, name="rec")
                nc.vector.reciprocal(rec[:], ssq[:])
                nc.vector.tensor_scalar(ab_sb[:], rec[:], 0.5, None, OP.mult)
                nc.vector.tensor_copy(abH_sb[:], ab_sb[:])
                nc.vector.tensor_tensor(abL_sb[:], ab_sb[:], abH_sb[:],
                                        OP.subtract)
                nc.vector.tensor_copy(F2r_sb[:], F2_sb[:])
                logs = post.tile([128, 128], F32, name="logs")
                nc.scalar.activation(logs[:], s12[:], AF.Ln)
                lr = logs.rearrange("p (x two) -> p x two", two=2)
                t1 = post.tile([128, 64], F32, name="t1")
                nc.vector.tensor_tensor(t1[:], lr[:, :, 0], lr[:, :, 1], OP.add)
                nc.vector.tensor_scalar(lc_sb[:], t1[:], LOG2PI, -1.0,
                                        OP.add, OP.mult)

            # ---- Phase E: attention + target main loop ----
            with (
                tc.tile_pool(name="mp", bufs=1) as mp,
                tc.tile_pool(name="mps", bufs=1, space="PSUM") as mps,
            ):
                for hp in range(4):
                    h0, h1 = 2 * hp, 2 * hp + 1
                    # attn scores in [l, s] layout -> E (with Z accum) -> p
                    for lt in range(8):
                        lsl = slice(lt * 128, (lt + 1) * 128)
                        for h in (h0, h1):
                            rsl = slice((h % 2) * 64, (h % 2) * 64 + 64)
                            zc = h * 8 + lt
                            at2 = mps.tile([128, L], F32, name="scoreps2",
                                           tag="scoreps", bufs=2)
                            for half in range(2):
                                hsl = slice(half * 512, (half + 1) * 512)
                                nc.tensor.matmul(
                                    at2[:, hsl], qpH[hp][rsl, lsl],
                                    kpH[hp][rsl, hsl], start=True, stop=False)
                                nc.tensor.matmul(
                                    at2[:, hsl], qpH[hp][rsl, lsl],
                                    kpL[hp][rsl, hsl], start=False, stop=False)
                                nc.tensor.matmul(
                                    at2[:, hsl], qpL[hp][rsl, lsl],
                                    kpH[hp][rsl, hsl], start=False, stop=True)
                            e2 = mp.tile([128, L], F32, name="Esb", tag="Esb",
                                         bufs=3)
                            nc.scalar.activation(
                                e2[:], at2[:], AF.Exp, scale=SCALE,
                                accum_out=z_all[:, zc:zc + 1])
                            nc.vector.reciprocal(
                                rz_sb[:, zc:zc + 1], z_all[:, zc:zc + 1])
                            nc.vector.tensor_scalar(
                                e2[:], e2[:], rz_sb[:, zc:zc + 1], None, OP.mult)
                            nc.sync.dma_start(p_ap[h, lsl, :], e2[:])
                    # transposed scores -> ET, then out = E @ v per l-tile
                    for h in (h0, h1):
                        rsl = slice((h % 2) * 64, (h % 2) * 64 + 64)
                        et = {}
                        for st in range(8):
                            ssl = slice(st * 128, (st + 1) * 128)
                            at = mps.tile([128, L], F32, name="scoreps",
                                          tag="scoreps", bufs=2)
                            for half in range(2):
                                hsl = slice(half * 512, (half + 1) * 512)
                                nc.tensor.matmul(
                                    at[:, hsl], kpH[hp][rsl, ssl],
                                    qpH[hp][rsl, hsl], start=True, stop=False)
                                nc.tensor.matmul(
                                    at[:, hsl], kpH[hp][rsl, ssl],
                                    qpL[hp][rsl, hsl], start=False, stop=False)
                                nc.tensor.matmul(
                                    at[:, hsl], kpL[hp][rsl, ssl],
                                    qpH[hp][rsl, hsl], start=False, stop=True)
                            e = mp.tile([128, L], F32, name="ET", tag="ET",
                                        bufs=10)
                            nc.scalar.activation(e[:], at[:], AF.Exp, scale=SCALE)
                            et[st] = e
                        for lt in range(8):
                            zc = h * 8 + lt
                            ops_ = mps.tile([128, 64], F32, name="outps",
                                            tag="outps", bufs=2)
                            for st in range(8):
                                nc.tensor.matmul(
                                    ops_[:], et[st][:, lt * 128:(lt + 1) * 128],
                                    v_sb[st][:, h * 64:(h + 1) * 64],
                                    start=(st == 0), stop=(st == 7))
                            nc.vector.tensor_scalar(
                                osb[lt][:, h * 64:(h + 1) * 64], ops_[:],
                                rz_sb[:, zc:zc + 1], None, OP.mult)
                    # target tiles for both heads.  Exponent
                    # u = a*dy + b*dx as ONE K=128 float32r matmul with
                    # exact hi/lo mantissa splitting:
                    # G cols [0:32]=Y*a_hi [32:64]=X*b_hi [64:96]=Y*a_lo
                    # [96:128]=X*b_lo ; F2 rows = [Fy; Fx; Fy; Fx].
                    for lt in range(8):
                        lsl = slice(lt * 128, (lt + 1) * 128)
                        ysl = slice(lt * NV, (lt + 1) * NV)
                        for h in (h0, h1):
                            ac = lt * 16 + 2 * h
                            Gc = mp.tile([128, 128], F32, name="Gc", tag="Gc",
                                         bufs=2)
                            abHf = abH_sb.bitcast(F32)
                            abLf = abL_sb.bitcast(F32)
                            nc.vector.tensor_scalar(
                                Gc[:, 0:32], Y_sb[:, ysl],
                                abHf[:, ac:ac + 1], None, OP.mult)
                            nc.vector.tensor_scalar(
                                Gc[:, 32:64], X_sb[:, ysl],
                                abHf[:, ac + 1:ac + 2], None, OP.mult)
                            nc.vector.tensor_scalar(
                                Gc[:, 64:96], Y_sb[:, ysl],
                                abLf[:, ac:ac + 1], None, OP.mult)
                            nc.vector.tensor_scalar(
                                Gc[:, 96:128], X_sb[:, ysl],
                                abLf[:, ac + 1:ac + 2], None, OP.mult)
                            gt_ps = mps.tile([128, 128], F32, name="gt_ps",
                                             tag="ups", bufs=2)
                            nc.tensor.transpose(gt_ps[:], Gc[:], id_sb[:])
                            GTr = mp.tile([128, 128], F32R, name="GTr",
                                          tag="GTr", bufs=2)
                            nc.vector.tensor_copy(GTr[:], gt_ps[:])
                            tt = mp.tile([128, L], F32, name="tsb", tag="tsb",
                                         bufs=3)
                            for half in range(2):
                                hsl = slice(half * 512, (half + 1) * 512)
                                ups = mps.tile([128, 512], F32, name="ups",
                                               tag="ups", bufs=2)
                                nc.tensor.matmul(
                                    ups[:], GTr[:], F2r_sb[:, hsl],
                                    start=True, stop=True)
                                nc.scalar.activation(
                                    tt[:, hsl], ups[:], AF.Exp, scale=-1.0,
                                    bias=lc_sb[:, lt * 8 + h:lt * 8 + h + 1])
                            nc.sync.dma_start(t_ap[h, lsl, :], tt[:])

            for lt in range(8):
                nc.sync.dma_start(out_ap[lt * 128:(lt + 1) * 128, :], osb[lt][:])

    nc.compile()
    return nc


def _dedup_rows(dmat):
    """Decompose dmat [L, L] into (labels [L], reps [G, L]) with
    dmat[l, :] == reps[labels[l], :] exactly. Returns None if > NV groups."""
    uniq, inv = np.unique(dmat, axis=0, return_inverse=True)
    if uniq.shape[0] > NV:
        return None
    return inv.astype(np.int64), uniq


def _host_fallback(query, key, value, distances_x, distances_y,
                   Wq, bq, Wk, bk, Wv, bv, Wsig, bsig):
    """Pure-numpy reference (used only if the distance matrices are not
    decomposable into <=32 row groups per axis — never for the real task)."""
    b, l, d_model = query.shape
    d = d_model // H
    scale = 1.0 / math.sqrt(d)
    sig = (query @ Wsig.T + bsig).reshape(b, l, H, 2).transpose(0, 2, 1, 3)
    sig = 1.0 / (1.0 + np.exp(-sig * 5.0)) + 1e-5
    sig = np.power(3.0, sig) - 1.0
    s1 = sig[..., 0]
    s2 = sig[..., 1]
    target = (1.0 / (2.0 * math.pi * s1 * s2))[..., None] * np.exp(
        -distances_y[None, None] / (2.0 * (s1 ** 2))[..., None]
        - distances_x[None, None] / (2.0 * (s2 ** 2))[..., None])
    q = (query @ Wq.T + bq).reshape(b, l, H, d)
    k = (key @ Wk.T + bk).reshape(b, l, H, d)
    v = (value @ Wv.T + bv).reshape(b, l, H, d)
    attn = scale * np.einsum('blhe,bshe->bhls', q, k)
    attn = attn - attn.max(axis=-1, keepdims=True)
    p = np.exp(attn)
    p = p / p.sum(axis=-1, keepdims=True)
    out = np.einsum('bhls,bshd->blhd', p, v).reshape(b, l, d_model)
    return (out.astype(np.float32), p.astype(np.float32),
            target.astype(np.float32))


def kernel(query, key, value, distances_x, distances_y,
           Wq, bq, Wk, bk, Wv, bv, Wsig, bsig, _trace=False):
    global last_results
    f = np.float32
    query = np.ascontiguousarray(query, dtype=f)
    key = np.ascontiguousarray(key, dtype=f)
    value = np.ascontiguousarray(value, dtype=f)
    dy = np.ascontiguousarray(distances_y, dtype=f)
    dx = np.ascontiguousarray(distances_x, dtype=f)

    dy_dec = _dedup_rows(dy)
    dx_dec = _dedup_rows(dx)
    if dy_dec is None or dx_dec is None:
        return _host_fallback(query, key, value, dx, dy, Wq, bq, Wk, bk,
                              Wv, bv, Wsig, bsig)
    ylab, yrep = dy_dec
    xlab, xrep = dx_dec

    # Indicator masks [L, NV] rearranged to [128, 8*NV] (l-tile-major cols)
    def _mask(lab):
        m = np.zeros((L, NV), dtype=f)
        m[np.arange(L), lab] = 1.0
        return np.ascontiguousarray(
            m.reshape(8, 128, NV).transpose(1, 0, 2).reshape(128, 8 * NV))

    Ym, Xm = _mask(ylab), _mask(xlab)
    Fy = np.zeros((NV, L), dtype=f)
    Fy[:yrep.shape[0]] = yrep
    Fx = np.zeros((NV, L), dtype=f)
    Fx[:xrep.shape[0]] = xrep
    F1 = np.concatenate([Fy, Fx], axis=0)          # [64, L]
    F2 = np.ascontiguousarray(np.concatenate([F1, F1], axis=0))  # [128, L]

    shared = {
        "WqT": np.ascontiguousarray(np.asarray(Wq, f).T),
        "WkT": np.ascontiguousarray(np.asarray(Wk, f).T),
        "WvT": np.ascontiguousarray(np.asarray(Wv, f).T),
        "WsigT": np.ascontiguousarray(np.asarray(Wsig, f).T),
        "bqT": np.ascontiguousarray(np.asarray(bq, f).reshape(4, 128).T),
        "bkT": np.ascontiguousarray(np.asarray(bk, f).reshape(4, 128).T),
        "bvR": np.ascontiguousarray(np.asarray(bv, f).reshape(1, D)),
        "bsR": np.ascontiguousarray(np.asarray(bsig, f).reshape(1, 2 * H)),
        "Ymask": Ym, "Xmask": Xm, "F2": F2,
        "ident": np.eye(128, dtype=f),
    }

    if "nc" not in _module_cache:
        _module_cache["nc"] = _build_module()
    nc = _module_cache["nc"]

    in_maps = []
    for b in range(N_CORES):
        m = dict(shared)
        m["query"] = query[b]
        m["key"] = key[b]
        m["value"] = value[b]
        in_maps.append(m)

    res = run_bass_kernel_spmd(nc, in_maps, core_ids=list(range(N_CORES)),
                               trace=_trace)
    last_results = res

    out = np.stack([res.results[b]["out"] for b in range(N_CORES)])
    p = np.stack([res.results[b]["p"] for b in range(N_CORES)])
    target = np.stack([res.results[b]["target"] for b in range(N_CORES)])
    return out, p, target


# revision 9
# speedup vs baseline: 1.0806x; 1.0806x over previous
"""Trainium2 Bass kernel for nn_Attention2D (sparse_attention).

Computes, per batch element b (data-parallel over 8 NeuronCores):
  sig    = query @ Wsig.T + bsig -> per-head Gaussian widths (s1, s2)
  target = 1/(2*pi*s1*s2) * exp(-dy/(2*s1^2) - dx/(2*s2^2))   [H, L, L]
  q,k,v  = projections; attn = softmax(q k^T / sqrt(dh))      [H, L, L]
  out    = attn @ v                                            [L, D]

Key device-side structure per core:
  - query/key/value transposed on PE (identity matmuls) to feed projections.
  - q_projT / k_projT kept head-major on partitions so K=64 attention
    matmuls for even/odd heads land on disjoint PE row groups (row tiling).
  - scores computed twice (attn [l,s] and attnT [s,l]); exp on ACT gives
    E (for p, with free running-sum accum_out = softmax denominator Z) and
    ET (transposed weights feeding the out = E @ v matmul as lhsT).
  - target exponent u = a_l*dy + b_l*dx realized exactly as a K=64 matmul
    against indicator-selected rows of dy/dx (dedup of dy rows, host-side),
    avoiding any catastrophic cancellation for huge a (up to ~4e9).
"""

import math
import os
import sys

import numpy as np

for _p in ("/opt/trn_rl_repo", "/root/.axon_site/_ro/trn_rl_repo"):
    if os.path.isdir(_p) and _p not in sys.path:
        sys.path.insert(0, _p)

import concourse.bacc as bacc
import concourse.mybir as mybir
import concourse.tile as tile
from concourse.bass_utils import run_bass_kernel_spmd

F32 = mybir.dt.float32
F32R = mybir.dt.float32r
AF = mybir.ActivationFunctionType
OP = mybir.AluOpType

B, L, D, H = 8, 1024, 512, 8
DH = D // H            # 64
NV = 32                # max distinct dy/dx row groups per axis
SCALE = 1.0 / math.sqrt(DH)
LN3 = math.log(3.0)
LOG2PI = math.log(2.0 * math.pi)
N_CORES = 8

_module_cache = {}
last_results = None  # BassKernelResults of the most recent device run


def _build_module():
    nc = bacc.Bacc("TRN2", target_bir_lowering=False, debug=False)

    q_ap = nc.dram_tensor("query", [L, D], F32, kind="ExternalInput").ap()
    k_ap = nc.dram_tensor("key", [L, D], F32, kind="ExternalInput").ap()
    v_ap = nc.dram_tensor("value", [L, D], F32, kind="ExternalInput").ap()
    wqT_ap = nc.dram_tensor("WqT", [D, D], F32, kind="ExternalInput").ap()
    wkT_ap = nc.dram_tensor("WkT", [D, D], F32, kind="ExternalInput").ap()
    wvT_ap = nc.dram_tensor("WvT", [D, D], F32, kind="ExternalInput").ap()
    wsT_ap = nc.dram_tensor("WsigT", [D, 2 * H], F32, kind="ExternalInput").ap()
    bq_ap = nc.dram_tensor("bqT", [128, 4], F32, kind="ExternalInput").ap()
    bk_ap = nc.dram_tensor("bkT", [128, 4], F32, kind="ExternalInput").ap()
    bv_ap = nc.dram_tensor("bvR", [1, D], F32, kind="ExternalInput").ap()
    bs_ap = nc.dram_tensor("bsR", [1, 2 * H], F32, kind="ExternalInput").ap()
    Y_ap = nc.dram_tensor("Ymask", [128, 8 * NV], F32, kind="ExternalInput").ap()
    X_ap = nc.dram_tensor("Xmask", [128, 8 * NV], F32, kind="ExternalInput").ap()
    F2_ap = nc.dram_tensor("F2", [128, L], F32, kind="ExternalInput").ap()
    id_ap = nc.dram_tensor("ident", [128, 128], F32, kind="ExternalInput").ap()

    out_ap = nc.dram_tensor("out", [L, D], F32, kind="ExternalOutput").ap()
    p_ap = nc.dram_tensor("p", [H, L, L], F32, kind="ExternalOutput").ap()
    t_ap = nc.dram_tensor("target", [H, L, L], F32, kind="ExternalOutput").ap()

    with tile.TileContext(nc) as tc:
        with tc.tile_pool(name="pers", bufs=1) as pers:
            qpT = [pers.tile([128, L], F32, name=f"qpT{c}") for c in range(4)]
            kpT = [pers.tile([128, L], F32, name=f"kpT{c}") for c in range(4)]
            v_sb = [pers.tile([128, D], F32, name=f"vsb{st}") for st in range(8)]
            osb = [pers.tile([128, D], F32, name=f"osb{lt}") for lt in range(8)]
            F2r_sb = pers.tile([128, L], F32R, name="F2r_sb")
            sig_sb = pers.tile([128, 128], F32, name="sig_sb")
            ab_sb = pers.tile([128, 128], F32, name="ab_sb")
            abH_sb = pers.tile([128, 128], F32R, name="abH_sb")
            abL_sb = pers.tile([128, 128], F32R, name="abL_sb")
            lc_sb = pers.tile([128, 64], F32, name="lc_sb")
            z_all = pers.tile([128, 64], F32, name="z_all")
            rz_sb = pers.tile([128, 64], F32, name="rz_sb")
            Y_sb = pers.tile([128, 8 * NV], F32, name="Y_sb")
            X_sb = pers.tile([128, 8 * NV], F32, name="X_sb")
            F2_sb = pers.tile([128, L], F32, name="F2_sb")
            id_sb = pers.tile([128, 128], F32, name="id_sb")
            ones_sb = pers.tile([1, 128], F32, name="ones_sb")
            bq_sb = pers.tile([128, 4], F32, name="bq_sb")
            bk_sb = pers.tile([128, 4], F32, name="bk_sb")
            bv_sb = pers.tile([1, D], F32, name="bv_sb")
            bs_sb = pers.tile([1, 2 * H], F32, name="bs_sb")

            nc.sync.dma_start(Y_sb[:], Y_ap[:])
            nc.sync.dma_start(X_sb[:], X_ap[:])
            nc.sync.dma_start(F2_sb[:], F2_ap[:])
            nc.sync.dma_start(id_sb[:], id_ap[:])
            nc.sync.dma_start(bq_sb[:], bq_ap[:])
            nc.sync.dma_start(bk_sb[:], bk_ap[:])
            nc.sync.dma_start(bv_sb[:], bv_ap[:])
            nc.sync.dma_start(bs_sb[:], bs_ap[:])
            nc.gpsimd.memset(ones_sb[:], 1.0)
            nc.gpsimd.memset(z_all[:], 0.0)

            # ---- Phase A+B: input transposes + projections ----
            with (
                tc.tile_pool(name="pab", bufs=1) as pab,
                tc.tile_pool(name="psab", bufs=2, space="PSUM") as psab,
            ):
                wq_sb = [pab.tile([128, D], F32, name=f"wq{kc}") for kc in range(4)]
                wk_sb = [pab.tile([128, D], F32, name=f"wk{kc}") for kc in range(4)]
                wv_sb = [pab.tile([128, D], F32, name=f"wv{kc}") for kc in range(4)]
                ws_sb = [pab.tile([128, 2 * H], F32, name=f"ws{kc}") for kc in range(4)]
                for kc in range(4):
                    sl = slice(kc * 128, (kc + 1) * 128)
                    nc.sync.dma_start(wq_sb[kc][:], wqT_ap[sl, :])
                    nc.sync.dma_start(wk_sb[kc][:], wkT_ap[sl, :])
                    nc.sync.dma_start(wv_sb[kc][:], wvT_ap[sl, :])
                    nc.sync.dma_start(ws_sb[kc][:], wsT_ap[sl, :])

                xT = {}
                for nm in ("q", "k", "v"):
                    for kc in range(4):
                        xT[(nm, kc)] = pab.tile([128, L], F32, name=f"{nm}T{kc}")
                for nm, ap in (("q", q_ap), ("k", k_ap), ("v", v_ap)):
                    for lt in range(8):
                        raw = pab.tile([128, D], F32, name="raw", tag="raw", bufs=3)
                        nc.sync.dma_start(raw[:], ap[lt * 128:(lt + 1) * 128, :])
                        tp = psab.tile([128, 512], F32, name="tp", tag="tp", bufs=2)
                        for kc in range(4):
                            nc.tensor.transpose(
                                tp[:, kc * 128:(kc + 1) * 128],
                                raw[:, kc * 128:(kc + 1) * 128],
                                id_sb[:],
                            )
                        for kc in range(4):
                            nc.vector.tensor_copy(
                                xT[(nm, kc)][:, lt * 128:(lt + 1) * 128],
                                tp[:, kc * 128:(kc + 1) * 128],
                            )

                # q_projT / k_projT: [d_out, l], head-major on partitions
                for c in range(4):
                    for half in range(2):
                        hsl = slice(half * 512, (half + 1) * 512)
                        pq = psab.tile([128, 512], F32, name="pq", tag="pq", bufs=2)
                        for kc in range(4):
                            nc.tensor.matmul(
                                pq[:], wq_sb[kc][:, c * 128:(c + 1) * 128],
                                xT[("q", kc)][:, hsl],
                                start=(kc == 0), stop=(kc == 3),
                            )
                        nc.vector.tensor_scalar(
                            qpT[c][:, hsl], pq[:], bq_sb[:, c:c + 1], None, OP.add)
                        pk = psab.tile([128, 512], F32, name="pk", tag="pq", bufs=2)
                        for kc in range(4):
                            nc.tensor.matmul(
                                pk[:], wk_sb[kc][:, c * 128:(c + 1) * 128],
                                xT[("k", kc)][:, hsl],
                                start=(kc == 0), stop=(kc == 3),
                            )
                        nc.vector.tensor_scalar(
                            kpT[c][:, hsl], pk[:], bk_sb[:, c:c + 1], None, OP.add)

                # v_proj: natural [s, d] layout (+ bias via rank-1 matmul)
                for st in range(8):
                    pv = psab.tile([128, 512], F32, name="pv", tag="pq", bufs=2)
                    for kc in range(4):
                        nc.tensor.matmul(
                            pv[:], xT[("v", kc)][:, st * 128:(st + 1) * 128],
                            wv_sb[kc][:], start=(kc == 0), stop=False,
                        )
                    nc.tensor.matmul(pv[:], ones_sb[:1, :], bv_sb[:1, :],
                                     start=False, stop=True)
                    nc.vector.tensor_copy(v_sb[st][:], pv[:])

                # sig = query @ Wsig.T + bsig: [l, 16] per l-tile
                for lt in range(8):
                    psg = psab.tile([128, 2 * H], F32, name="psg", tag="psg", bufs=2)
                    for kc in range(4):
                        nc.tensor.matmul(
                            psg[:], xT[("q", kc)][:, lt * 128:(lt + 1) * 128],
                            ws_sb[kc][:], start=(kc == 0), stop=False,
                        )
                    nc.tensor.matmul(psg[:], ones_sb[:1, :], bs_sb[:1, :],
                                     start=False, stop=True)
                    nc.vector.tensor_copy(
                        sig_sb[:, lt * 16:(lt + 1) * 16], psg[:])

            # ---- Phase C: sigma transforms ----
            # s = 3^(sigmoid(5x)+1e-5) - 1 ; a/b = 1/(2 s^2) ;
            # logc = -(log(2 pi) + log s1 + log s2)
            with tc.tile_pool(name="post", bufs=1) as post:
                sg = post.tile([128, 128], F32, name="sg")
                nc.scalar.activation(sg[:], sig_sb[:], AF.Sigmoid, scale=5.0)
                # s = expm1(z), z = ln3*(sigmoid+1e-5).  exp(z)-1 cancels
                # catastrophically for tiny z (s ~ 1e-5 drives the largest
                # target values), so blend a Taylor branch below z=0.12.
                zt = post.tile([128, 128], F32, name="zt")
                nc.vector.tensor_scalar(zt[:], sg[:], LN3, LN3 * 1e-5,
                                        OP.mult, OP.add)
                ez = post.tile([128, 128], F32, name="ez")
                nc.scalar.activation(ez[:], zt[:], AF.Exp)
                s12 = post.tile([128, 128], F32, name="s12")
                nc.vector.tensor_scalar(s12[:], ez[:], -1.0, None, OP.add)
                hh = post.tile([128, 128], F32, name="hh")
                tmp = post.tile([128, 128], F32, name="tmp")
                nc.vector.tensor_scalar(hh[:], zt[:], 1.0 / 120, 1.0 / 24,
                                        OP.mult, OP.add)
                for cst in (1.0 / 6, 0.5, 1.0):
                    nc.vector.tensor_tensor(tmp[:], hh[:], zt[:], OP.mult)
                    nc.vector.tensor_scalar(hh[:], tmp[:], cst, None, OP.add)
                st_ = post.tile([128, 128], F32, name="st_")
                nc.vector.tensor_tensor(st_[:], hh[:], zt[:], OP.mult)
                msk = post.tile([128, 128], mybir.dt.uint8, name="msk")
                nc.vector.tensor_scalar(msk[:], zt[:], 0.12, None, OP.is_lt)
                nc.vector.copy_predicated(s12[:], msk[:], st_[:])
                ssq = post.tile([128, 128], F32, name="ssq")
                nc.vector.tensor_tensor(ssq[:], s12[:], s12[:], OP.mult)
                rec = post.tile([128, 128], F32, name="rec")
                nc.vector.reciprocal(rec[:], ssq[:])
                nc.vector.tensor_scalar(ab_sb[:], rec[:], 0.5, None, OP.mult)
                nc.vector.tensor_copy(abH_sb[:], ab_sb[:])
                nc.vector.tensor_tensor(abL_sb[:], ab_sb[:], abH_sb[:],
                                        OP.subtract)
                nc.vector.tensor_copy(F2r_sb[:], F2_sb[:])
                logs = post.tile([128, 128], F32, name="logs")
                nc.scalar.activation(logs[:], s12[:], AF.Ln)
                lr = logs.rearrange("p (x two) -> p x two", two=2)
                t1 = post.tile([128, 64], F32, name="t1")
                nc.vector.tensor_tensor(t1[:], lr[:, :, 0], lr[:, :, 1], OP.add)
                nc.vector.tensor_scalar(lc_sb[:], t1[:], LOG2PI, -1.0,
                                        OP.add, OP.mult)

            # ---- Phase E: attention + target main loop ----
            with (
                tc.tile_pool(name="mp", bufs=1) as mp,
                tc.tile_pool(name="mps", bufs=1, space="PSUM") as mps,
            ):
                for hp in range(4):
                    h0, h1 = 2 * hp, 2 * hp + 1
                    # attn scores in [l, s] layout -> E (with Z accum) -> p
                    for lt in range(8):
                        lsl = slice(lt * 128, (lt + 1) * 128)
                        for h in (h0, h1):
                            rsl = slice((h % 2) * 64, (h % 2) * 64 + 64)
                            zc = h * 8 + lt
                            at2 = mps.tile([128, L], F32, name="scoreps2",
                                           tag="scoreps", bufs=2)
                            for half in range(2):
                                hsl = slice(half * 512, (half + 1) * 512)
                                nc.tensor.matmul(
                                    at2[:, hsl], qpT[hp][rsl, lsl],
                                    kpT[hp][rsl, hsl], start=True, stop=True)
                            e2 = mp.tile([128, L], F32, name="Esb", tag="Esb",
                                         bufs=3)
                            nc.scalar.activation(
                                e2[:], at2[:], AF.Exp, scale=SCALE,
                                accum_out=z_all[:, zc:zc + 1])
                            nc.vector.reciprocal(
                                rz_sb[:, zc:zc + 1], z_all[:, zc:zc + 1])
                            nc.vector.tensor_scalar(
                                e2[:], e2[:], rz_sb[:, zc:zc + 1], None, OP.mult)
                            nc.sync.dma_start(p_ap[h, lsl, :], e2[:])
                    # transposed scores -> ET, then out = E @ v per l-tile
                    for h in (h0, h1):
                        rsl = slice((h % 2) * 64, (h % 2) * 64 + 64)
                        et = {}
                        for st in range(8):
                            ssl = slice(st * 128, (st + 1) * 128)
                            at = mps.tile([128, L], F32, name="scoreps",
                                          tag="scoreps", bufs=2)
                            for half in range(2):
                                hsl = slice(half * 512, (half + 1) * 512)
                                nc.tensor.matmul(
                                    at[:, hsl], kpT[hp][rsl, ssl],
                                    qpT[hp][rsl, hsl], start=True, stop=True)
                            e = mp.tile([128, L], F32, name="ET", tag="ET",
                                        bufs=10)
                            nc.scalar.activation(e[:], at[:], AF.Exp, scale=SCALE)
                            et[st] = e
                        for lt in range(8):
                            zc = h * 8 + lt
                            ops_ = mps.tile([128, 64], F32, name="outps",
                                            tag="outps", bufs=2)
                            for st in range(8):
                                nc.tensor.matmul(
                                    ops_[:], et[st][:, lt * 128:(lt + 1) * 128],
                                    v_sb[st][:, h * 64:(h + 1) * 64],
                                    start=(st == 0), stop=(st == 7))
                            nc.vector.tensor_scalar(
                                osb[lt][:, h * 64:(h + 1) * 64], ops_[:],
                                rz_sb[:, zc:zc + 1], None, OP.mult)
                    # target tiles for both heads.  Exponent
                    # u = a*dy + b*dx as ONE K=128 float32r matmul with
                    # exact hi/lo mantissa splitting:
                    # G cols [0:32]=Y*a_hi [32:64]=X*b_hi [64:96]=Y*a_lo
                    # [96:128]=X*b_lo ; F2 rows = [Fy; Fx; Fy; Fx].
                    for lt in range(8):
                        lsl = slice(lt * 128, (lt + 1) * 128)
                        ysl = slice(lt * NV, (lt + 1) * NV)
                        for h in (h0, h1):
                            ac = lt * 16 + 2 * h
                            Gc = mp.tile([128, 128], F32, name="Gc", tag="Gc",
                                         bufs=2)
                            abHf = abH_sb.bitcast(F32)
                            abLf = abL_sb.bitcast(F32)
                            nc.vector.tensor_scalar(
                                Gc[:, 0:32], Y_sb[:, ysl],
                                abHf[:, ac:ac + 1], None, OP.mult)
                            nc.vector.tensor_scalar(
                                Gc[:, 32:64], X_sb[:, ysl],
                                abHf[:, ac + 1:ac + 2], None, OP.mult)
                            nc.vector.tensor_scalar(
                                Gc[:, 64:96], Y_sb[:, ysl],
                                abLf[:, ac:ac + 1], None, OP.mult)
                            nc.vector.tensor_scalar(
                                Gc[:, 96:128], X_sb[:, ysl],
                                abLf[:, ac + 1:ac + 2], None, OP.mult)
                            gt_ps = mps.tile([128, 128], F32, name="gt_ps",
                                             tag="ups", bufs=2)
                            nc.tensor.transpose(gt_ps[:], Gc[:], id_sb[:])
                            GTr = mp.tile([128, 128], F32R, name="GTr",
                                          tag="GTr", bufs=2)
                            nc.vector.tensor_copy(GTr[:], gt_ps[:])
                            tt = mp.tile([128, L], F32, name="tsb", tag="tsb",
                                         bufs=3)
                            for half in range(2):
                                hsl = slice(half * 512, (half + 1) * 512)
                                ups = mps.tile([128, 512], F32, name="ups",
                                               tag="ups", bufs=2)
                                nc.tensor.matmul(
                                    ups[:], GTr[:], F2r_sb[:, hsl],
                                    start=True, stop=True)
                                nc.scalar.activation(
                                    tt[:, hsl], ups[:], AF.Exp, scale=-1.0,
                                    bias=lc_sb[:, lt * 8 + h:lt * 8 + h + 1])
                            nc.sync.dma_start(t_ap[h, lsl, :], tt[:])

            for lt in range(8):
                nc.sync.dma_start(out_ap[lt * 128:(lt + 1) * 128, :], osb[lt][:])

    nc.compile()
    return nc


def _dedup_rows(dmat):
    """Decompose dmat [L, L] into (labels [L], reps [G, L]) with
    dmat[l, :] == reps[labels[l], :] exactly. Returns None if > NV groups."""
    uniq, inv = np.unique(dmat, axis=0, return_inverse=True)
    if uniq.shape[0] > NV:
        return None
    return inv.astype(np.int64), uniq


def _host_fallback(query, key, value, distances_x, distances_y,
                   Wq, bq, Wk, bk, Wv, bv, Wsig, bsig):
    """Pure-numpy reference (used only if the distance matrices are not
    decomposable into <=32 row groups per axis — never for the real task)."""
    b, l, d_model = query.shape
    d = d_model // H
    scale = 1.0 / math.sqrt(d)
    sig = (query @ Wsig.T + bsig).reshape(b, l, H, 2).transpose(0, 2, 1, 3)
    sig = 1.0 / (1.0 + np.exp(-sig * 5.0)) + 1e-5
    sig = np.power(3.0, sig) - 1.0
    s1 = sig[..., 0]
    s2 = sig[..., 1]
    target = (1.0 / (2.0 * math.pi * s1 * s2))[..., None] * np.exp(
        -distances_y[None, None] / (2.0 * (s1 ** 2))[..., None]
        - distances_x[None, None] / (2.0 * (s2 ** 2))[..., None])
    q = (query @ Wq.T + bq).reshape(b, l, H, d)
    k = (key @ Wk.T + bk).reshape(b, l, H, d)
    v = (value @ Wv.T + bv).reshape(b, l, H, d)
    attn = scale * np.einsum('blhe,bshe->bhls', q, k)
    attn = attn - attn.max(axis=-1, keepdims=True)
    p = np.exp(attn)
    p = p / p.sum(axis=-1, keepdims=True)
    out = np.einsum('bhls,bshd->blhd', p, v).reshape(b, l, d_model)
    return (out.astype(np.float32), p.astype(np.float32),
            target.astype(np.float32))


def kernel(query, key, value, distances_x, distances_y,
           Wq, bq, Wk, bk, Wv, bv, Wsig, bsig, _trace=False):
    global last_results
    f = np.float32
    query = np.ascontiguousarray(query, dtype=f)
    key = np.ascontiguousarray(key, dtype=f)
    value = np.ascontiguousarray(value, dtype=f)
    dy = np.ascontiguousarray(distances_y, dtype=f)
    dx = np.ascontiguousarray(distances_x, dtype=f)

    dy_dec = _dedup_rows(dy)
    dx_dec = _dedup_rows(dx)
    if dy_dec is None or dx_dec is None:
        return _host_fallback(query, key, value, dx, dy, Wq, bq, Wk, bk,
                              Wv, bv, Wsig, bsig)
    ylab, yrep = dy_dec
    xlab, xrep = dx_dec

    # Indicator masks [L, NV] rearranged to [128, 8*NV] (l-tile-major cols)
    def _mask(lab):
        m = np.zeros((L, NV), dtype=f)
        m[np.arange(L), lab] = 1.0
        return np.ascontiguousarray(
            m.reshape(8, 128, NV).transpose(1, 0, 2).reshape(128, 8 * NV))

    Ym, Xm = _mask(ylab), _mask(xlab)
    Fy = np.zeros((NV, L), dtype=f)
    Fy[:yrep.shape[0]] = yrep
    Fx = np.zeros((NV, L), dtype=f)
    Fx[:xrep.shape[0]] = xrep
    F1 = np.concatenate([Fy, Fx], axis=0)          # [64, L]
    F2 = np.ascontiguousarray(np.concatenate([F1, F1], axis=0))  # [128, L]

    shared = {
        "WqT": np.ascontiguousarray(np.asarray(Wq, f).T),
        "WkT": np.ascontiguousarray(np.asarray(Wk, f).T),
        "WvT": np.ascontiguousarray(np.asarray(Wv, f).T),
        "WsigT": np.ascontiguousarray(np.asarray(Wsig, f).T),
        "bqT": np.ascontiguousarray(np.asarray(bq, f).reshape(4, 128).T),
        "bkT": np.ascontiguousarray(np.asarray(bk, f).reshape(4, 128).T),
        "bvR": np.ascontiguousarray(np.asarray(bv, f).reshape(1, D)),
        "bsR": np.ascontiguousarray(np.asarray(bsig, f).reshape(1, 2 * H)),
        "Ymask": Ym, "Xmask": Xm, "F2": F2,
        "ident": np.eye(128, dtype=f),
    }

    if "nc" not in _module_cache:
        _module_cache["nc"] = _build_module()
    nc = _module_cache["nc"]

    in_maps = []
    for b in range(N_CORES):
        m = dict(shared)
        m["query"] = query[b]
        m["key"] = key[b]
        m["value"] = value[b]
        in_maps.append(m)

    res = run_bass_kernel_spmd(nc, in_maps, core_ids=list(range(N_CORES)),
                               trace=_trace)
    last_results = res

    out = np.stack([res.results[b]["out"] for b in range(N_CORES)])
    p = np.stack([res.results[b]["p"] for b in range(N_CORES)])
    target = np.stack([res.results[b]["target"] for b in range(N_CORES)])
    return out, p, target


# revision 10
# speedup vs baseline: 1.2044x; 1.1146x over previous
"""Trainium2 Bass kernel for nn_Attention2D (sparse_attention).

Computes, per batch element b (data-parallel over 8 NeuronCores):
  sig    = query @ Wsig.T + bsig -> per-head Gaussian widths (s1, s2)
  target = 1/(2*pi*s1*s2) * exp(-dy/(2*s1^2) - dx/(2*s2^2))   [H, L, L]
  q,k,v  = projections; attn = softmax(q k^T / sqrt(dh))      [H, L, L]
  out    = attn @ v                                            [L, D]

Key device-side structure per core:
  - query/key/value transposed on PE (identity matmuls) to feed projections.
  - q_projT / k_projT kept head-major on partitions so K=64 attention
    matmuls for even/odd heads land on disjoint PE row groups (row tiling).
  - scores computed twice (attn [l,s] and attnT [s,l]); exp on ACT gives
    E (for p, with free running-sum accum_out = softmax denominator Z) and
    ET (transposed weights feeding the out = E @ v matmul as lhsT).
  - target exponent u = a_l*dy + b_l*dx realized exactly as a K=64 matmul
    against indicator-selected rows of dy/dx (dedup of dy rows, host-side),
    avoiding any catastrophic cancellation for huge a (up to ~4e9).
"""

import math
import os
import sys

import numpy as np

for _p in ("/opt/trn_rl_repo", "/root/.axon_site/_ro/trn_rl_repo"):
    if os.path.isdir(_p) and _p not in sys.path:
        sys.path.insert(0, _p)

import concourse.bacc as bacc
import concourse.mybir as mybir
import concourse.tile as tile
from concourse.bass_utils import run_bass_kernel_spmd

F32 = mybir.dt.float32
F32R = mybir.dt.float32r
AF = mybir.ActivationFunctionType
OP = mybir.AluOpType

B, L, D, H = 8, 1024, 512, 8
DH = D // H            # 64
NV = 32                # max distinct dy/dx row groups per axis
SCALE = 1.0 / math.sqrt(DH)
LN3 = math.log(3.0)
LOG2PI = math.log(2.0 * math.pi)
N_CORES = 8

_module_cache = {}
last_results = None  # BassKernelResults of the most recent device run


def _build_module():
    nc = bacc.Bacc("TRN2", target_bir_lowering=False, debug=False)

    q_ap = nc.dram_tensor("query", [L, D], F32, kind="ExternalInput").ap()
    k_ap = nc.dram_tensor("key", [L, D], F32, kind="ExternalInput").ap()
    v_ap = nc.dram_tensor("value", [L, D], F32, kind="ExternalInput").ap()
    wqT_ap = nc.dram_tensor("WqT", [D, D], F32, kind="ExternalInput").ap()
    wkT_ap = nc.dram_tensor("WkT", [D, D], F32, kind="ExternalInput").ap()
    wvT_ap = nc.dram_tensor("WvT", [D, D], F32, kind="ExternalInput").ap()
    wsT_ap = nc.dram_tensor("WsigT", [D, 2 * H], F32, kind="ExternalInput").ap()
    bq_ap = nc.dram_tensor("bqT", [128, 4], F32, kind="ExternalInput").ap()
    bk_ap = nc.dram_tensor("bkT", [128, 4], F32, kind="ExternalInput").ap()
    bv_ap = nc.dram_tensor("bvR", [1, D], F32, kind="ExternalInput").ap()
    bs_ap = nc.dram_tensor("bsR", [1, 2 * H], F32, kind="ExternalInput").ap()
    Y_ap = nc.dram_tensor("Ymask", [128, 8 * NV], F32, kind="ExternalInput").ap()
    X_ap = nc.dram_tensor("Xmask", [128, 8 * NV], F32, kind="ExternalInput").ap()
    F2_ap = nc.dram_tensor("F2", [128, L], F32, kind="ExternalInput").ap()
    id_ap = nc.dram_tensor("ident", [128, 128], F32, kind="ExternalInput").ap()

    out_ap = nc.dram_tensor("out", [L, D], F32, kind="ExternalOutput").ap()
    p_ap = nc.dram_tensor("p", [H, L, L], F32, kind="ExternalOutput").ap()
    t_ap = nc.dram_tensor("target", [H, L, L], F32, kind="ExternalOutput").ap()

    with tile.TileContext(nc) as tc:
        with tc.tile_pool(name="pers", bufs=1) as pers:
            qpT = [pers.tile([128, L], F32, name=f"qpT{c}") for c in range(4)]
            kpT = [pers.tile([128, L], F32, name=f"kpT{c}") for c in range(4)]
            v_sb = [pers.tile([128, D], F32, name=f"vsb{st}") for st in range(8)]
            osb = [pers.tile([128, D], F32, name=f"osb{lt}") for lt in range(8)]
            F2r_sb = pers.tile([128, L], F32R, name="F2r_sb")
            sig_sb = pers.tile([128, 128], F32, name="sig_sb")
            ab_sb = pers.tile([128, 128], F32, name="ab_sb")
            abH_sb = pers.tile([128, 128], F32R, name="abH_sb")
            abL_sb = pers.tile([128, 128], F32R, name="abL_sb")
            lc_sb = pers.tile([128, 64], F32, name="lc_sb")
            z_all = pers.tile([128, 64], F32, name="z_all")
            rz_sb = pers.tile([128, 64], F32, name="rz_sb")
            Y_sb = pers.tile([128, 8 * NV], F32, name="Y_sb")
            X_sb = pers.tile([128, 8 * NV], F32, name="X_sb")
            F2_sb = pers.tile([128, L], F32, name="F2_sb")
            id_sb = pers.tile([128, 128], F32, name="id_sb")
            ones_sb = pers.tile([1, 128], F32, name="ones_sb")
            bq_sb = pers.tile([128, 4], F32, name="bq_sb")
            bk_sb = pers.tile([128, 4], F32, name="bk_sb")
            bv_sb = pers.tile([1, D], F32, name="bv_sb")
            bs_sb = pers.tile([1, 2 * H], F32, name="bs_sb")

            nc.sync.dma_start(Y_sb[:], Y_ap[:])
            nc.sync.dma_start(X_sb[:], X_ap[:])
            nc.sync.dma_start(F2_sb[:], F2_ap[:])
            nc.sync.dma_start(id_sb[:], id_ap[:])
            nc.sync.dma_start(bq_sb[:], bq_ap[:])
            nc.sync.dma_start(bk_sb[:], bk_ap[:])
            nc.sync.dma_start(bv_sb[:], bv_ap[:])
            nc.sync.dma_start(bs_sb[:], bs_ap[:])
            nc.gpsimd.memset(ones_sb[:], 1.0)
            nc.gpsimd.memset(z_all[:], 0.0)

            # ---- Phase A+B: input transposes + projections ----
            with (
                tc.tile_pool(name="pab", bufs=1) as pab,
                tc.tile_pool(name="psab", bufs=2, space="PSUM") as psab,
            ):
                wq_sb = [pab.tile([128, D], F32, name=f"wq{kc}") for kc in range(4)]
                wk_sb = [pab.tile([128, D], F32, name=f"wk{kc}") for kc in range(4)]
                wv_sb = [pab.tile([128, D], F32, name=f"wv{kc}") for kc in range(4)]
                ws_sb = [pab.tile([128, 2 * H], F32, name=f"ws{kc}") for kc in range(4)]
                for kc in range(4):
                    sl = slice(kc * 128, (kc + 1) * 128)
                    nc.sync.dma_start(wq_sb[kc][:], wqT_ap[sl, :])
                    nc.sync.dma_start(wk_sb[kc][:], wkT_ap[sl, :])
                    nc.sync.dma_start(wv_sb[kc][:], wvT_ap[sl, :])
                    nc.sync.dma_start(ws_sb[kc][:], wsT_ap[sl, :])

                xT = {}
                for nm in ("q", "k", "v"):
                    for kc in range(4):
                        xT[(nm, kc)] = pab.tile([128, L], F32, name=f"{nm}T{kc}")
                for nm, ap in (("q", q_ap), ("k", k_ap), ("v", v_ap)):
                    for lt in range(8):
                        raw = pab.tile([128, D], F32, name="raw", tag="raw", bufs=3)
                        nc.sync.dma_start(raw[:], ap[lt * 128:(lt + 1) * 128, :])
                        tp = psab.tile([128, 512], F32, name="tp", tag="tp", bufs=2)
                        for kc in range(4):
                            nc.tensor.transpose(
                                tp[:, kc * 128:(kc + 1) * 128],
                                raw[:, kc * 128:(kc + 1) * 128],
                                id_sb[:],
                            )
                        for kc in range(4):
                            nc.vector.tensor_copy(
                                xT[(nm, kc)][:, lt * 128:(lt + 1) * 128],
                                tp[:, kc * 128:(kc + 1) * 128],
                            )

                # q_projT / k_projT: [d_out, l], head-major on partitions
                for c in range(4):
                    for half in range(2):
                        hsl = slice(half * 512, (half + 1) * 512)
                        pq = psab.tile([128, 512], F32, name="pq", tag="pq", bufs=2)
                        for kc in range(4):
                            nc.tensor.matmul(
                                pq[:], wq_sb[kc][:, c * 128:(c + 1) * 128],
                                xT[("q", kc)][:, hsl],
                                start=(kc == 0), stop=(kc == 3),
                            )
                        nc.vector.tensor_scalar(
                            qpT[c][:, hsl], pq[:], bq_sb[:, c:c + 1], None, OP.add)
                        pk = psab.tile([128, 512], F32, name="pk", tag="pq", bufs=2)
                        for kc in range(4):
                            nc.tensor.matmul(
                                pk[:], wk_sb[kc][:, c * 128:(c + 1) * 128],
                                xT[("k", kc)][:, hsl],
                                start=(kc == 0), stop=(kc == 3),
                            )
                        nc.vector.tensor_scalar(
                            kpT[c][:, hsl], pk[:], bk_sb[:, c:c + 1], None, OP.add)

                # v_proj: natural [s, d] layout (+ bias via rank-1 matmul)
                for st in range(8):
                    pv = psab.tile([128, 512], F32, name="pv", tag="pq", bufs=2)
                    for kc in range(4):
                        nc.tensor.matmul(
                            pv[:], xT[("v", kc)][:, st * 128:(st + 1) * 128],
                            wv_sb[kc][:], start=(kc == 0), stop=False,
                        )
                    nc.tensor.matmul(pv[:], ones_sb[:1, :], bv_sb[:1, :],
                                     start=False, stop=True)
                    nc.vector.tensor_copy(v_sb[st][:], pv[:])

                # sig = query @ Wsig.T + bsig: [l, 16] per l-tile
                for lt in range(8):
                    psg = psab.tile([128, 2 * H], F32, name="psg", tag="psg", bufs=2)
                    for kc in range(4):
                        nc.tensor.matmul(
                            psg[:], xT[("q", kc)][:, lt * 128:(lt + 1) * 128],
                            ws_sb[kc][:], start=(kc == 0), stop=False,
                        )
                    nc.tensor.matmul(psg[:], ones_sb[:1, :], bs_sb[:1, :],
                                     start=False, stop=True)
                    nc.vector.tensor_copy(
                        sig_sb[:, lt * 16:(lt + 1) * 16], psg[:])

            # ---- Phase C: sigma transforms ----
            # s = 3^(sigmoid(5x)+1e-5) - 1 ; a/b = 1/(2 s^2) ;
            # logc = -(log(2 pi) + log s1 + log s2)
            with tc.tile_pool(name="post", bufs=1) as post:
                sg = post.tile([128, 128], F32, name="sg")
                nc.scalar.activation(sg[:], sig_sb[:], AF.Sigmoid, scale=5.0)
                # s = expm1(z), z = ln3*(sigmoid+1e-5).  exp(z)-1 cancels
                # catastrophically for tiny z (s ~ 1e-5 drives the largest
                # target values), so blend a Taylor branch below z=0.12.
                zt = post.tile([128, 128], F32, name="zt")
                nc.vector.tensor_scalar(zt[:], sg[:], LN3, LN3 * 1e-5,
                                        OP.mult, OP.add)
                ez = post.tile([128, 128], F32, name="ez")
                nc.scalar.activation(ez[:], zt[:], AF.Exp)
                s12 = post.tile([128, 128], F32, name="s12")
                nc.vector.tensor_scalar(s12[:], ez[:], -1.0, None, OP.add)
                hh = post.tile([128, 128], F32, name="hh")
                tmp = post.tile([128, 128], F32, name="tmp")
                nc.vector.tensor_scalar(hh[:], zt[:], 1.0 / 120, 1.0 / 24,
                                        OP.mult, OP.add)
                for cst in (1.0 / 6, 0.5, 1.0):
                    nc.vector.tensor_tensor(tmp[:], hh[:], zt[:], OP.mult)
                    nc.vector.tensor_scalar(hh[:], tmp[:], cst, None, OP.add)
                st_ = post.tile([128, 128], F32, name="st_")
                nc.vector.tensor_tensor(st_[:], hh[:], zt[:], OP.mult)
                msk = post.tile([128, 128], mybir.dt.uint8, name="msk")
                nc.vector.tensor_scalar(msk[:], zt[:], 0.12, None, OP.is_lt)
                nc.vector.copy_predicated(s12[:], msk[:], st_[:])
                ssq = post.tile([128, 128], F32, name="ssq")
                nc.vector.tensor_tensor(ssq[:], s12[:], s12[:], OP.mult)
                rec = post.tile([128, 128], F32, name="rec")
                nc.vector.reciprocal(rec[:], ssq[:])
                nc.vector.tensor_scalar(ab_sb[:], rec[:], 0.5, None, OP.mult)
                nc.vector.tensor_copy(abH_sb[:], ab_sb[:])
                nc.vector.tensor_tensor(abL_sb[:], ab_sb[:], abH_sb[:],
                                        OP.subtract)
                nc.vector.tensor_copy(F2r_sb[:], F2_sb[:])
                logs = post.tile([128, 128], F32, name="logs")
                nc.scalar.activation(logs[:], s12[:], AF.Ln)
                lr = logs.rearrange("p (x two) -> p x two", two=2)
                t1 = post.tile([128, 64], F32, name="t1")
                nc.vector.tensor_tensor(t1[:], lr[:, :, 0], lr[:, :, 1], OP.add)
                nc.vector.tensor_scalar(lc_sb[:], t1[:], LOG2PI, -1.0,
                                        OP.add, OP.mult)

            # ---- Phase E: attention + target main loop ----
            with (
                tc.tile_pool(name="mp", bufs=1) as mp,
                tc.tile_pool(name="mps", bufs=1, space="PSUM") as mps,
            ):
                for hp in range(4):
                    h0, h1 = 2 * hp, 2 * hp + 1
                    # attn scores in [l, s] layout -> E (with Z accum) -> p
                    for lt in range(8):
                        lsl = slice(lt * 128, (lt + 1) * 128)
                        for h in (h0, h1):
                            rsl = slice((h % 2) * 64, (h % 2) * 64 + 64)
                            zc = h * 8 + lt
                            at2 = mps.tile([128, L], F32, name="scoreps2",
                                           tag="scoreps", bufs=2)
                            for half in range(2):
                                hsl = slice(half * 512, (half + 1) * 512)
                                nc.tensor.matmul(
                                    at2[:, hsl], qpT[hp][rsl, lsl],
                                    kpT[hp][rsl, hsl], start=True, stop=True)
                            e2 = mp.tile([128, L], F32, name="Esb", tag="Esb",
                                         bufs=3)
                            nc.scalar.activation(
                                e2[:], at2[:], AF.Exp, scale=SCALE,
                                accum_out=z_all[:, zc:zc + 1])
                            nc.vector.reciprocal(
                                rz_sb[:, zc:zc + 1], z_all[:, zc:zc + 1])
                            nc.vector.tensor_scalar(
                                e2[:], e2[:], rz_sb[:, zc:zc + 1], None, OP.mult)
                            nc.sync.dma_start(p_ap[h, lsl, :], e2[:])
                    # transposed scores -> ET; outT = v.T @ ET (v stationary,
                    # 512-wide moving operand), then PE-transpose outT back.
                    for h in (h0, h1):
                        rsl = slice((h % 2) * 64, (h % 2) * 64 + 64)
                        oT = mps.tile([64, L], F32, name="outTps",
                                      tag="outTps", bufs=1)
                        for st in range(8):
                            ssl = slice(st * 128, (st + 1) * 128)
                            at = mps.tile([128, L], F32, name="scoreps",
                                          tag="scoreps", bufs=2)
                            for half in range(2):
                                hsl = slice(half * 512, (half + 1) * 512)
                                nc.tensor.matmul(
                                    at[:, hsl], kpT[hp][rsl, ssl],
                                    qpT[hp][rsl, hsl], start=True, stop=True)
                            e = mp.tile([128, L], F32, name="ET", tag="ET",
                                        bufs=4)
                            nc.scalar.activation(e[:], at[:], AF.Exp, scale=SCALE)
                            for half in range(2):
                                hsl = slice(half * 512, (half + 1) * 512)
                                nc.tensor.matmul(
                                    oT[:, hsl], v_sb[st][:, h * 64:(h + 1) * 64],
                                    e[:, hsl], start=(st == 0), stop=(st == 7))
                        oTs = mp.tile([64, L], F32, name="oTs", tag="oTs",
                                      bufs=2)
                        nc.vector.tensor_copy(oTs[:], oT[:])
                        for lt in range(8):
                            zc = h * 8 + lt
                            otp = mps.tile([128, 64], F32, name="otp",
                                           tag="ups", bufs=2)
                            nc.tensor.transpose(
                                otp[:], oTs[:, lt * 128:(lt + 1) * 128],
                                id_sb[0:64, 0:64])
                            nc.vector.tensor_scalar(
                                osb[lt][:, h * 64:(h + 1) * 64], otp[:],
                                rz_sb[:, zc:zc + 1], None, OP.mult)
                    # target tiles for both heads.  Exponent
                    # u = a*dy + b*dx as ONE K=128 float32r matmul with
                    # exact hi/lo mantissa splitting:
                    # G cols [0:32]=Y*a_hi [32:64]=X*b_hi [64:96]=Y*a_lo
                    # [96:128]=X*b_lo ; F2 rows = [Fy; Fx; Fy; Fx].
                    for lt in range(8):
                        lsl = slice(lt * 128, (lt + 1) * 128)
                        ysl = slice(lt * NV, (lt + 1) * NV)
                        for h in (h0, h1):
                            ac = lt * 16 + 2 * h
                            Gc = mp.tile([128, 128], F32, name="Gc", tag="Gc",
                                         bufs=2)
                            abHf = abH_sb.bitcast(F32)
                            abLf = abL_sb.bitcast(F32)
                            nc.vector.tensor_scalar(
                                Gc[:, 0:32], Y_sb[:, ysl],
                                abHf[:, ac:ac + 1], None, OP.mult)
                            nc.vector.tensor_scalar(
                                Gc[:, 32:64], X_sb[:, ysl],
                                abHf[:, ac + 1:ac + 2], None, OP.mult)
                            nc.vector.tensor_scalar(
                                Gc[:, 64:96], Y_sb[:, ysl],
                                abLf[:, ac:ac + 1], None, OP.mult)
                            nc.vector.tensor_scalar(
                                Gc[:, 96:128], X_sb[:, ysl],
                                abLf[:, ac + 1:ac + 2], None, OP.mult)
                            gt_ps = mps.tile([128, 128], F32, name="gt_ps",
                                             tag="ups", bufs=2)
                            nc.tensor.transpose(gt_ps[:], Gc[:], id_sb[:])
                            GTr = mp.tile([128, 128], F32R, name="GTr",
                                          tag="GTr", bufs=2)
                            nc.vector.tensor_copy(GTr[:], gt_ps[:])
                            tt = mp.tile([128, L], F32, name="tsb", tag="tsb",
                                         bufs=3)
                            for half in range(2):
                                hsl = slice(half * 512, (half + 1) * 512)
                                ups = mps.tile([128, 512], F32, name="ups",
                                               tag="ups", bufs=2)
                                nc.tensor.matmul(
                                    ups[:], GTr[:], F2r_sb[:, hsl],
                                    start=True, stop=True)
                                nc.scalar.activation(
                                    tt[:, hsl], ups[:], AF.Exp, scale=-1.0,
                                    bias=lc_sb[:, lt * 8 + h:lt * 8 + h + 1])
                            nc.sync.dma_start(t_ap[h, lsl, :], tt[:])

            for lt in range(8):
                nc.sync.dma_start(out_ap[lt * 128:(lt + 1) * 128, :], osb[lt][:])

    nc.compile()
    return nc


def _dedup_rows(dmat):
    """Decompose dmat [L, L] into (labels [L], reps [G, L]) with
    dmat[l, :] == reps[labels[l], :] exactly. Returns None if > NV groups."""
    uniq, inv = np.unique(dmat, axis=0, return_inverse=True)
    if uniq.shape[0] > NV:
        return None
    return inv.astype(np.int64), uniq


def _host_fallback(query, key, value, distances_x, distances_y,
                   Wq, bq, Wk, bk, Wv, bv, Wsig, bsig):
    """Pure-numpy reference (used only if the distance matrices are not
    decomposable into <=32 row groups per axis — never for the real task)."""
    b, l, d_model = query.shape
    d = d_model // H
    scale = 1.0 / math.sqrt(d)
    sig = (query @ Wsig.T + bsig).reshape(b, l, H, 2).transpose(0, 2, 1, 3)
    sig = 1.0 / (1.0 + np.exp(-sig * 5.0)) + 1e-5
    sig = np.power(3.0, sig) - 1.0
    s1 = sig[..., 0]
    s2 = sig[..., 1]
    target = (1.0 / (2.0 * math.pi * s1 * s2))[..., None] * np.exp(
        -distances_y[None, None] / (2.0 * (s1 ** 2))[..., None]
        - distances_x[None, None] / (2.0 * (s2 ** 2))[..., None])
    q = (query @ Wq.T + bq).reshape(b, l, H, d)
    k = (key @ Wk.T + bk).reshape(b, l, H, d)
    v = (value @ Wv.T + bv).reshape(b, l, H, d)
    attn = scale * np.einsum('blhe,bshe->bhls', q, k)
    attn = attn - attn.max(axis=-1, keepdims=True)
    p = np.exp(attn)
    p = p / p.sum(axis=-1, keepdims=True)
    out = np.einsum('bhls,bshd->blhd', p, v).reshape(b, l, d_model)
    return (out.astype(np.float32), p.astype(np.float32),
            target.astype(np.float32))


def kernel(query, key, value, distances_x, distances_y,
           Wq, bq, Wk, bk, Wv, bv, Wsig, bsig, _trace=False):
    global last_results
    f = np.float32
    query = np.ascontiguousarray(query, dtype=f)
    key = np.ascontiguousarray(key, dtype=f)
    value = np.ascontiguousarray(value, dtype=f)
    dy = np.ascontiguousarray(distances_y, dtype=f)
    dx = np.ascontiguousarray(distances_x, dtype=f)

    dy_dec = _dedup_rows(dy)
    dx_dec = _dedup_rows(dx)
    if dy_dec is None or dx_dec is None:
        return _host_fallback(query, key, value, dx, dy, Wq, bq, Wk, bk,
                              Wv, bv, Wsig, bsig)
    ylab, yrep = dy_dec
    xlab, xrep = dx_dec

    # Indicator masks [L, NV] rearranged to [128, 8*NV] (l-tile-major cols)
    def _mask(lab):
        m = np.zeros((L, NV), dtype=f)
        m[np.arange(L), lab] = 1.0
        return np.ascontiguousarray(
            m.reshape(8, 128, NV).transpose(1, 0, 2).reshape(128, 8 * NV))

    Ym, Xm = _mask(ylab), _mask(xlab)
    Fy = np.zeros((NV, L), dtype=f)
    Fy[:yrep.shape[0]] = yrep
    Fx = np.zeros((NV, L), dtype=f)
    Fx[:xrep.shape[0]] = xrep
    F1 = np.concatenate([Fy, Fx], axis=0)          # [64, L]
    F2 = np.ascontiguousarray(np.concatenate([F1, F1], axis=0))  # [128, L]

    shared = {
        "WqT": np.ascontiguousarray(np.asarray(Wq, f).T),
        "WkT": np.ascontiguousarray(np.asarray(Wk, f).T),
        "WvT": np.ascontiguousarray(np.asarray(Wv, f).T),
        "WsigT": np.ascontiguousarray(np.asarray(Wsig, f).T),
        "bqT": np.ascontiguousarray(np.asarray(bq, f).reshape(4, 128).T),
        "bkT": np.ascontiguousarray(np.asarray(bk, f).reshape(4, 128).T),
        "bvR": np.ascontiguousarray(np.asarray(bv, f).reshape(1, D)),
        "bsR": np.ascontiguousarray(np.asarray(bsig, f).reshape(1, 2 * H)),
        "Ymask": Ym, "Xmask": Xm, "F2": F2,
        "ident": np.eye(128, dtype=f),
    }

    if "nc" not in _module_cache:
        _module_cache["nc"] = _build_module()
    nc = _module_cache["nc"]

    in_maps = []
    for b in range(N_CORES):
        m = dict(shared)
        m["query"] = query[b]
        m["key"] = key[b]
        m["value"] = value[b]
        in_maps.append(m)

    res = run_bass_kernel_spmd(nc, in_maps, core_ids=list(range(N_CORES)),
                               trace=_trace)
    last_results = res

    out = np.stack([res.results[b]["out"] for b in range(N_CORES)])
    p = np.stack([res.results[b]["p"] for b in range(N_CORES)])
    target = np.stack([res.results[b]["target"] for b in range(N_CORES)])
    return out, p, target


# revision 11
# speedup vs baseline: 1.2795x; 1.0623x over previous
"""Trainium2 Bass kernel for nn_Attention2D (sparse_attention).

Computes, per batch element b (data-parallel over 8 NeuronCores):
  sig    = query @ Wsig.T + bsig -> per-head Gaussian widths (s1, s2)
  target = 1/(2*pi*s1*s2) * exp(-dy/(2*s1^2) - dx/(2*s2^2))   [H, L, L]
  q,k,v  = projections; attn = softmax(q k^T / sqrt(dh))      [H, L, L]
  out    = attn @ v                                            [L, D]

Key device-side structure per core:
  - query/key/value transposed on PE (identity matmuls) to feed projections.
  - q_projT / k_projT kept head-major on partitions so K=64 attention
    matmuls for even/odd heads land on disjoint PE row groups (row tiling).
  - scores computed twice (attn [l,s] and attnT [s,l]); exp on ACT gives
    E (for p, with free running-sum accum_out = softmax denominator Z) and
    ET (transposed weights feeding the out = E @ v matmul as lhsT).
  - target exponent u = a_l*dy + b_l*dx realized exactly as a K=64 matmul
    against indicator-selected rows of dy/dx (dedup of dy rows, host-side),
    avoiding any catastrophic cancellation for huge a (up to ~4e9).
"""

import math
import os
import sys

import numpy as np

for _p in ("/opt/trn_rl_repo", "/root/.axon_site/_ro/trn_rl_repo"):
    if os.path.isdir(_p) and _p not in sys.path:
        sys.path.insert(0, _p)

import concourse.bacc as bacc
import concourse.mybir as mybir
import concourse.tile as tile
from concourse.bass_utils import run_bass_kernel_spmd

F32 = mybir.dt.float32
F32R = mybir.dt.float32r
AF = mybir.ActivationFunctionType
OP = mybir.AluOpType

B, L, D, H = 8, 1024, 512, 8
DH = D // H            # 64
NV = 32                # max distinct dy/dx row groups per axis
SCALE = 1.0 / math.sqrt(DH)
LN3 = math.log(3.0)
LOG2PI = math.log(2.0 * math.pi)
N_CORES = 8

_module_cache = {}
last_results = None  # BassKernelResults of the most recent device run


def _build_module():
    nc = bacc.Bacc("TRN2", target_bir_lowering=False, debug=False)

    q_ap = nc.dram_tensor("query", [L, D], F32, kind="ExternalInput").ap()
    k_ap = nc.dram_tensor("key", [L, D], F32, kind="ExternalInput").ap()
    v_ap = nc.dram_tensor("value", [L, D], F32, kind="ExternalInput").ap()
    wqT_ap = nc.dram_tensor("WqT", [D, D], F32, kind="ExternalInput").ap()
    wkT_ap = nc.dram_tensor("WkT", [D, D], F32, kind="ExternalInput").ap()
    wvT_ap = nc.dram_tensor("WvT", [D, D], F32, kind="ExternalInput").ap()
    wsT_ap = nc.dram_tensor("WsigT", [D, 2 * H], F32, kind="ExternalInput").ap()
    bq_ap = nc.dram_tensor("bqT", [128, 4], F32, kind="ExternalInput").ap()
    bk_ap = nc.dram_tensor("bkT", [128, 4], F32, kind="ExternalInput").ap()
    bv_ap = nc.dram_tensor("bvR", [1, D], F32, kind="ExternalInput").ap()
    bs_ap = nc.dram_tensor("bsR", [1, 2 * H], F32, kind="ExternalInput").ap()
    Y_ap = nc.dram_tensor("Ymask", [128, 8 * NV], F32, kind="ExternalInput").ap()
    X_ap = nc.dram_tensor("Xmask", [128, 8 * NV], F32, kind="ExternalInput").ap()
    F2_ap = nc.dram_tensor("F2", [128, L], F32, kind="ExternalInput").ap()
    id_ap = nc.dram_tensor("ident", [128, 128], F32, kind="ExternalInput").ap()

    out_ap = nc.dram_tensor("out", [L, D], F32, kind="ExternalOutput").ap()
    p_ap = nc.dram_tensor("p", [H, L, L], F32, kind="ExternalOutput").ap()
    t_ap = nc.dram_tensor("target", [H, L, L], F32, kind="ExternalOutput").ap()

    with tile.TileContext(nc) as tc:
        with tc.tile_pool(name="pers", bufs=1) as pers:
            BF16 = mybir.dt.bfloat16
            qpH = [pers.tile([128, L], BF16, name=f"qpH{c}") for c in range(4)]
            qpL = [pers.tile([128, L], BF16, name=f"qpL{c}") for c in range(4)]
            kpH = [pers.tile([128, L], BF16, name=f"kpH{c}") for c in range(4)]
            kpL = [pers.tile([128, L], BF16, name=f"kpL{c}") for c in range(4)]
            v_sb = [pers.tile([128, D], F32, name=f"vsb{st}") for st in range(8)]
            osb = [pers.tile([128, D], F32, name=f"osb{lt}") for lt in range(8)]
            F2r_sb = pers.tile([128, L], F32R, name="F2r_sb")
            sig_sb = pers.tile([128, 128], F32, name="sig_sb")
            ab_sb = pers.tile([128, 128], F32, name="ab_sb")
            abH_sb = pers.tile([128, 128], F32R, name="abH_sb")
            abL_sb = pers.tile([128, 128], F32R, name="abL_sb")
            lc_sb = pers.tile([128, 64], F32, name="lc_sb")
            z_all = pers.tile([128, 64], F32, name="z_all")
            rz_sb = pers.tile([128, 64], F32, name="rz_sb")
            Y_sb = pers.tile([128, 8 * NV], F32, name="Y_sb")
            X_sb = pers.tile([128, 8 * NV], F32, name="X_sb")
            F2_sb = pers.tile([128, L], F32, name="F2_sb")
            id_sb = pers.tile([128, 128], F32, name="id_sb")
            ones_sb = pers.tile([1, 128], F32, name="ones_sb")
            bq_sb = pers.tile([128, 4], F32, name="bq_sb")
            bk_sb = pers.tile([128, 4], F32, name="bk_sb")
            bv_sb = pers.tile([1, D], F32, name="bv_sb")
            bs_sb = pers.tile([1, 2 * H], F32, name="bs_sb")

            nc.sync.dma_start(Y_sb[:], Y_ap[:])
            nc.sync.dma_start(X_sb[:], X_ap[:])
            nc.sync.dma_start(F2_sb[:], F2_ap[:])
            nc.sync.dma_start(id_sb[:], id_ap[:])
            nc.sync.dma_start(bq_sb[:], bq_ap[:])
            nc.sync.dma_start(bk_sb[:], bk_ap[:])
            nc.sync.dma_start(bv_sb[:], bv_ap[:])
            nc.sync.dma_start(bs_sb[:], bs_ap[:])
            nc.gpsimd.memset(ones_sb[:], 1.0)
            nc.gpsimd.memset(z_all[:], 0.0)

            # ---- Phase A+B: input transposes + projections ----
            with (
                tc.tile_pool(name="pab", bufs=1) as pab,
                tc.tile_pool(name="psab", bufs=2, space="PSUM") as psab,
            ):
                wq_sb = [pab.tile([128, D], F32, name=f"wq{kc}") for kc in range(4)]
                wk_sb = [pab.tile([128, D], F32, name=f"wk{kc}") for kc in range(4)]
                wv_sb = [pab.tile([128, D], F32, name=f"wv{kc}") for kc in range(4)]
                ws_sb = [pab.tile([128, 2 * H], F32, name=f"ws{kc}") for kc in range(4)]
                for kc in range(4):
                    sl = slice(kc * 128, (kc + 1) * 128)
                    nc.sync.dma_start(wq_sb[kc][:], wqT_ap[sl, :])
                    nc.sync.dma_start(wk_sb[kc][:], wkT_ap[sl, :])
                    nc.sync.dma_start(wv_sb[kc][:], wvT_ap[sl, :])
                    nc.sync.dma_start(ws_sb[kc][:], wsT_ap[sl, :])

                xT = {}
                for nm in ("q", "k", "v"):
                    for kc in range(4):
                        xT[(nm, kc)] = pab.tile([128, L], F32, name=f"{nm}T{kc}")
                for nm, ap in (("q", q_ap), ("k", k_ap), ("v", v_ap)):
                    for lt in range(8):
                        raw = pab.tile([128, D], F32, name="raw", tag="raw", bufs=3)
                        nc.sync.dma_start(raw[:], ap[lt * 128:(lt + 1) * 128, :])
                        tp = psab.tile([128, 512], F32, name="tp", tag="tp", bufs=2)
                        for kc in range(4):
                            nc.tensor.transpose(
                                tp[:, kc * 128:(kc + 1) * 128],
                                raw[:, kc * 128:(kc + 1) * 128],
                                id_sb[:],
                            )
                        for kc in range(4):
                            nc.vector.tensor_copy(
                                xT[(nm, kc)][:, lt * 128:(lt + 1) * 128],
                                tp[:, kc * 128:(kc + 1) * 128],
                            )

                # q_projT / k_projT: [d_out, l], head-major on partitions
                for c in range(4):
                    for half in range(2):
                        hsl = slice(half * 512, (half + 1) * 512)
                        pq = psab.tile([128, 512], F32, name="pq", tag="pq", bufs=2)
                        for kc in range(4):
                            nc.tensor.matmul(
                                pq[:], wq_sb[kc][:, c * 128:(c + 1) * 128],
                                xT[("q", kc)][:, hsl],
                                start=(kc == 0), stop=(kc == 3),
                            )
                        nc.vector.tensor_scalar(
                            qpH[c][:, hsl], pq[:], bq_sb[:, c:c + 1], None, OP.add)
                        nc.vector.scalar_tensor_tensor(
                            qpL[c][:, hsl], pq[:], bq_sb[:, c:c + 1],
                            qpH[c][:, hsl], OP.add, OP.subtract)
                        pk = psab.tile([128, 512], F32, name="pk", tag="pq", bufs=2)
                        for kc in range(4):
                            nc.tensor.matmul(
                                pk[:], wk_sb[kc][:, c * 128:(c + 1) * 128],
                                xT[("k", kc)][:, hsl],
                                start=(kc == 0), stop=(kc == 3),
                            )
                        nc.vector.tensor_scalar(
                            kpH[c][:, hsl], pk[:], bk_sb[:, c:c + 1], None, OP.add)
                        nc.vector.scalar_tensor_tensor(
                            kpL[c][:, hsl], pk[:], bk_sb[:, c:c + 1],
                            kpH[c][:, hsl], OP.add, OP.subtract)

                # v_proj: natural [s, d] layout (+ bias via rank-1 matmul)
                for st in range(8):
                    pv = psab.tile([128, 512], F32, name="pv", tag="pq", bufs=2)
                    for kc in range(4):
                        nc.tensor.matmul(
                            pv[:], xT[("v", kc)][:, st * 128:(st + 1) * 128],
                            wv_sb[kc][:], start=(kc == 0), stop=False,
                        )
                    nc.tensor.matmul(pv[:], ones_sb[:1, :], bv_sb[:1, :],
                                     start=False, stop=True)
                    nc.vector.tensor_copy(v_sb[st][:], pv[:])

                # sig = query @ Wsig.T + bsig: [l, 16] per l-tile
                for lt in range(8):
                    psg = psab.tile([128, 2 * H], F32, name="psg", tag="psg", bufs=2)
                    for kc in range(4):
                        nc.tensor.matmul(
                            psg[:], xT[("q", kc)][:, lt * 128:(lt + 1) * 128],
                            ws_sb[kc][:], start=(kc == 0), stop=False,
                        )
                    nc.tensor.matmul(psg[:], ones_sb[:1, :], bs_sb[:1, :],
                                     start=False, stop=True)
                    nc.vector.tensor_copy(
                        sig_sb[:, lt * 16:(lt + 1) * 16], psg[:])

            # ---- Phase C: sigma transforms ----
            # s = 3^(sigmoid(5x)+1e-5) - 1 ; a/b = 1/(2 s^2) ;
            # logc = -(log(2 pi) + log s1 + log s2)
            with tc.tile_pool(name="post", bufs=1) as post:
                sg = post.tile([128, 128], F32, name="sg")
                nc.scalar.activation(sg[:], sig_sb[:], AF.Sigmoid, scale=5.0)
                # s = expm1(z), z = ln3*(sigmoid+1e-5).  exp(z)-1 cancels
                # catastrophically for tiny z (s ~ 1e-5 drives the largest
                # target values), so blend a Taylor branch below z=0.12.
                zt = post.tile([128, 128], F32, name="zt")
                nc.vector.tensor_scalar(zt[:], sg[:], LN3, LN3 * 1e-5,
                                        OP.mult, OP.add)
                ez = post.tile([128, 128], F32, name="ez")
                nc.scalar.activation(ez[:], zt[:], AF.Exp)
                s12 = post.tile([128, 128], F32, name="s12")
                nc.vector.tensor_scalar(s12[:], ez[:], -1.0, None, OP.add)
                hh = post.tile([128, 128], F32, name="hh")
                tmp = post.tile([128, 128], F32, name="tmp")
                nc.vector.tensor_scalar(hh[:], zt[:], 1.0 / 120, 1.0 / 24,
                                        OP.mult, OP.add)
                for cst in (1.0 / 6, 0.5, 1.0):
                    nc.vector.tensor_tensor(tmp[:], hh[:], zt[:], OP.mult)
                    nc.vector.tensor_scalar(hh[:], tmp[:], cst, None, OP.add)
                st_ = post.tile([128, 128], F32, name="st_")
                nc.vector.tensor_tensor(st_[:], hh[:], zt[:], OP.mult)
                msk = post.tile([128, 128], mybir.dt.uint8, name="msk")
                nc.vector.tensor_scalar(msk[:], zt[:], 0.12, None, OP.is_lt)
                nc.vector.copy_predicated(s12[:], msk[:], st_[:])
                ssq = post.tile([128, 128], F32, name="ssq")
                nc.vector.tensor_tensor(ssq[:], s12[:], s12[:], OP.mult)
                rec = post.tile([128, 128], F32, name="rec")
                nc.vector.reciprocal(rec[:], ssq[:])
                nc.vector.tensor_scalar(ab_sb[:], rec[:], 0.5, None, OP.mult)
                nc.vector.tensor_copy(abH_sb[:], ab_sb[:])
                nc.vector.tensor_tensor(abL_sb[:], ab_sb[:], abH_sb[:],
                                        OP.subtract)
                nc.vector.tensor_copy(F2r_sb[:], F2_sb[:])
                logs = post.tile([128, 128], F32, name="logs")
                nc.scalar.activation(logs[:], s12[:], AF.Ln)
                lr = logs.rearrange("p (x two) -> p x two", two=2)
                t1 = post.tile([128, 64], F32, name="t1")
                nc.vector.tensor_tensor(t1[:], lr[:, :, 0], lr[:, :, 1], OP.add)
                nc.vector.tensor_scalar(lc_sb[:], t1[:], LOG2PI, -1.0,
                                        OP.add, OP.mult)

            # ---- Phase E: attention + target main loop ----
            with (
                tc.tile_pool(name="mp", bufs=1) as mp,
                tc.tile_pool(name="mps", bufs=1, space="PSUM") as mps,
            ):
                for hp in range(4):
                    h0, h1 = 2 * hp, 2 * hp + 1
                    # attn scores in [l, s] layout -> E (with Z accum) -> p
                    for lt in range(8):
                        lsl = slice(lt * 128, (lt + 1) * 128)
                        for h in (h0, h1):
                            rsl = slice((h % 2) * 64, (h % 2) * 64 + 64)
                            zc = h * 8 + lt
                            at2 = mps.tile([128, L], F32, name="scoreps2",
                                           tag="scoreps", bufs=2)
                            for half in range(2):
                                hsl = slice(half * 512, (half + 1) * 512)
                                nc.tensor.matmul(
                                    at2[:, hsl], qpH[hp][rsl, lsl],
                                    kpH[hp][rsl, hsl], start=True, stop=False)
                                nc.tensor.matmul(
                                    at2[:, hsl], qpH[hp][rsl, lsl],
                                    kpL[hp][rsl, hsl], start=False, stop=False)
                                nc.tensor.matmul(
                                    at2[:, hsl], qpL[hp][rsl, lsl],
                                    kpH[hp][rsl, hsl], start=False, stop=True)
                            e2 = mp.tile([128, L], F32, name="Esb", tag="Esb",
                                         bufs=3)
                            nc.scalar.activation(
                                e2[:], at2[:], AF.Exp, scale=SCALE,
                                accum_out=z_all[:, zc:zc + 1])
                            nc.vector.reciprocal(
                                rz_sb[:, zc:zc + 1], z_all[:, zc:zc + 1])
                            nc.vector.tensor_scalar(
                                e2[:], e2[:], rz_sb[:, zc:zc + 1], None, OP.mult)
                            nc.sync.dma_start(p_ap[h, lsl, :], e2[:])
                    # transposed scores -> ET; outT = v.T @ ET (v stationary,
                    # 512-wide moving operand), then PE-transpose outT back.
                    for h in (h0, h1):
                        rsl = slice((h % 2) * 64, (h % 2) * 64 + 64)
                        oT = mps.tile([64, L], F32, name="outTps",
                                      tag="outTps", bufs=1)
                        for st in range(8):
                            ssl = slice(st * 128, (st + 1) * 128)
                            at = mps.tile([128, L], F32, name="scoreps",
                                          tag="scoreps", bufs=2)
                            for half in range(2):
                                hsl = slice(half * 512, (half + 1) * 512)
                                nc.tensor.matmul(
                                    at[:, hsl], kpH[hp][rsl, ssl],
                                    qpH[hp][rsl, hsl], start=True, stop=False)
                                nc.tensor.matmul(
                                    at[:, hsl], kpH[hp][rsl, ssl],
                                    qpL[hp][rsl, hsl], start=False, stop=False)
                                nc.tensor.matmul(
                                    at[:, hsl], kpL[hp][rsl, ssl],
                                    qpH[hp][rsl, hsl], start=False, stop=True)
                            e = mp.tile([128, L], F32, name="ET", tag="ET",
                                        bufs=4)
                            nc.scalar.activation(e[:], at[:], AF.Exp, scale=SCALE)
                            for half in range(2):
                                hsl = slice(half * 512, (half + 1) * 512)
                                nc.tensor.matmul(
                                    oT[:, hsl], v_sb[st][:, h * 64:(h + 1) * 64],
                                    e[:, hsl], start=(st == 0), stop=(st == 7))
                        oTs = mp.tile([64, L], F32, name="oTs", tag="oTs",
                                      bufs=2)
                        nc.vector.tensor_copy(oTs[:], oT[:])
                        for lt in range(8):
                            zc = h * 8 + lt
                            otp = mps.tile([128, 64], F32, name="otp",
                                           tag="ups", bufs=2)
                            nc.tensor.transpose(
                                otp[:], oTs[:, lt * 128:(lt + 1) * 128],
                                id_sb[0:64, 0:64])
                            nc.vector.tensor_scalar(
                                osb[lt][:, h * 64:(h + 1) * 64], otp[:],
                                rz_sb[:, zc:zc + 1], None, OP.mult)
                    # target tiles for both heads.  Exponent
                    # u = a*dy + b*dx as ONE K=128 float32r matmul with
                    # exact hi/lo mantissa splitting:
                    # G cols [0:32]=Y*a_hi [32:64]=X*b_hi [64:96]=Y*a_lo
                    # [96:128]=X*b_lo ; F2 rows = [Fy; Fx; Fy; Fx].
                    for lt in range(8):
                        lsl = slice(lt * 128, (lt + 1) * 128)
                        ysl = slice(lt * NV, (lt + 1) * NV)
                        for h in (h0, h1):
                            ac = lt * 16 + 2 * h
                            Gc = mp.tile([128, 128], F32, name="Gc", tag="Gc",
                                         bufs=2)
                            abHf = abH_sb.bitcast(F32)
                            abLf = abL_sb.bitcast(F32)
                            nc.vector.tensor_scalar(
                                Gc[:, 0:32], Y_sb[:, ysl],
                                abHf[:, ac:ac + 1], None, OP.mult)
                            nc.vector.tensor_scalar(
                                Gc[:, 32:64], X_sb[:, ysl],
                                abHf[:, ac + 1:ac + 2], None, OP.mult)
                            nc.vector.tensor_scalar(
                                Gc[:, 64:96], Y_sb[:, ysl],
                                abLf[:, ac:ac + 1], None, OP.mult)
                            nc.vector.tensor_scalar(
                                Gc[:, 96:128], X_sb[:, ysl],
                                abLf[:, ac + 1:ac + 2], None, OP.mult)
                            gt_ps = mps.tile([128, 128], F32, name="gt_ps",
                                             tag="ups", bufs=2)
                            nc.tensor.transpose(gt_ps[:], Gc[:], id_sb[:])
                            GTr = mp.tile([128, 128], F32R, name="GTr",
                                          tag="GTr", bufs=2)
                            nc.vector.tensor_copy(GTr[:], gt_ps[:])
                            tt = mp.tile([128, L], F32, name="tsb", tag="tsb",
                                         bufs=3)
                            for half in range(2):
                                hsl = slice(half * 512, (half + 1) * 512)
                                ups = mps.tile([128, 512], F32, name="ups",
                                               tag="ups", bufs=2)
                                nc.tensor.matmul(
                                    ups[:], GTr[:], F2r_sb[:, hsl],
                                    start=True, stop=True)
                                nc.scalar.activation(
                                    tt[:, hsl], ups[:], AF.Exp, scale=-1.0,
                                    bias=lc_sb[:, lt * 8 + h:lt * 8 + h + 1])
                            nc.sync.dma_start(t_ap[h, lsl, :], tt[:])

            for lt in range(8):
                nc.sync.dma_start(out_ap[lt * 128:(lt + 1) * 128, :], osb[lt][:])

    nc.compile()
    return nc


def _dedup_rows(dmat):
    """Decompose dmat [L, L] into (labels [L], reps [G, L]) with
    dmat[l, :] == reps[labels[l], :] exactly. Returns None if > NV groups."""
    uniq, inv = np.unique(dmat, axis=0, return_inverse=True)
    if uniq.shape[0] > NV:
        return None
    return inv.astype(np.int64), uniq


def _host_fallback(query, key, value, distances_x, distances_y,
                   Wq, bq, Wk, bk, Wv, bv, Wsig, bsig):
    """Pure-numpy reference (used only if the distance matrices are not
    decomposable into <=32 row groups per axis — never for the real task)."""
    b, l, d_model = query.shape
    d = d_model // H
    scale = 1.0 / math.sqrt(d)
    sig = (query @ Wsig.T + bsig).reshape(b, l, H, 2).transpose(0, 2, 1, 3)
    sig = 1.0 / (1.0 + np.exp(-sig * 5.0)) + 1e-5
    sig = np.power(3.0, sig) - 1.0
    s1 = sig[..., 0]
    s2 = sig[..., 1]
    target = (1.0 / (2.0 * math.pi * s1 * s2))[..., None] * np.exp(
        -distances_y[None, None] / (2.0 * (s1 ** 2))[..., None]
        - distances_x[None, None] / (2.0 * (s2 ** 2))[..., None])
    q = (query @ Wq.T + bq).reshape(b, l, H, d)
    k = (key @ Wk.T + bk).reshape(b, l, H, d)
    v = (value @ Wv.T + bv).reshape(b, l, H, d)
    attn = scale * np.einsum('blhe,bshe->bhls', q, k)
    attn = attn - attn.max(axis=-1, keepdims=True)
    p = np.exp(attn)
    p = p / p.sum(axis=-1, keepdims=True)
    out = np.einsum('bhls,bshd->blhd', p, v).reshape(b, l, d_model)
    return (out.astype(np.float32), p.astype(np.float32),
            target.astype(np.float32))


def kernel(query, key, value, distances_x, distances_y,
           Wq, bq, Wk, bk, Wv, bv, Wsig, bsig, _trace=False):
    global last_results
    f = np.float32
    query = np.ascontiguousarray(query, dtype=f)
    key = np.ascontiguousarray(key, dtype=f)
    value = np.ascontiguousarray(value, dtype=f)
    dy = np.ascontiguousarray(distances_y, dtype=f)
    dx = np.ascontiguousarray(distances_x, dtype=f)

    dy_dec = _dedup_rows(dy)
    dx_dec = _dedup_rows(dx)
    if dy_dec is None or dx_dec is None:
        return _host_fallback(query, key, value, dx, dy, Wq, bq, Wk, bk,
                              Wv, bv, Wsig, bsig)
    ylab, yrep = dy_dec
    xlab, xrep = dx_dec

    # Indicator masks [L, NV] rearranged to [128, 8*NV] (l-tile-major cols)
    def _mask(lab):
        m = np.zeros((L, NV), dtype=f)
        m[np.arange(L), lab] = 1.0
        return np.ascontiguousarray(
            m.reshape(8, 128, NV).transpose(1, 0, 2).reshape(128, 8 * NV))

    Ym, Xm = _mask(ylab), _mask(xlab)
    Fy = np.zeros((NV, L), dtype=f)
    Fy[:yrep.shape[0]] = yrep
    Fx = np.zeros((NV, L), dtype=f)
    Fx[:xrep.shape[0]] = xrep
    F1 = np.concatenate([Fy, Fx], axis=0)          # [64, L]
    F2 = np.ascontiguousarray(np.concatenate([F1, F1], axis=0))  # [128, L]

    shared = {
        "WqT": np.ascontiguousarray(np.asarray(Wq, f).T),
        "WkT": np.ascontiguousarray(np.asarray(Wk, f).T),
        "WvT": np.ascontiguousarray(np.asarray(Wv, f).T),
        "WsigT": np.ascontiguousarray(np.asarray(Wsig, f).T),
        "bqT": np.ascontiguousarray(np.asarray(bq, f).reshape(4, 128).T),
        "bkT": np.ascontiguousarray(np.asarray(bk, f).reshape(4, 128).T),
        "bvR": np.ascontiguousarray(np.asarray(bv, f).reshape(1, D)),
        "bsR": np.ascontiguousarray(np.asarray(bsig, f).reshape(1, 2 * H)),
        "Ymask": Ym, "Xmask": Xm, "F2": F2,
        "ident": np.eye(128, dtype=f),
    }

    if "nc" not in _module_cache:
        _module_cache["nc"] = _build_module()
    nc = _module_cache["nc"]

    in_maps = []
    for b in range(N_CORES):
        m = dict(shared)
        m["query"] = query[b]
        m["key"] = key[b]
        m["value"] = value[b]
        in_maps.append(m)

    res = run_bass_kernel_spmd(nc, in_maps, core_ids=list(range(N_CORES)),
                               trace=_trace)
    last_results = res

    out = np.stack([res.results[b]["out"] for b in range(N_CORES)])
    p = np.stack([res.results[b]["p"] for b in range(N_CORES)])
    target = np.stack([res.results[b]["target"] for b in range(N_CORES)])
    return out, p, target
